# revision 14
# baseline (speedup 1.0000x reference)
"""Two-block transformer encoder (B=4, S=2048, D=256, H=8, DFF=1024) on 8
Trainium2 NeuronCores.

Sharding: core c -> batch b = c//2, sequence half = c%2 (1024 tokens owned).
Weights replicated. Block 1 computes K/V over the full sequence and
Q/FFN/LN over the owned half. Between blocks the halves are exchanged
with chunked AllGathers (bf16) that overlap block-2's Q projection.

v2 changes vs baseline:
- everything bf16 (activations, weights, exchange); PSUM stays fp32
- attention loop per (head-quad, qt): 4 score matmuls -> one 2048-wide
  exp on the scalar engine -> 4 PV matmuls; PV emission skewed one kc
  behind scores so the PE never stalls waiting for exp (strict FIFO)
- optional fp8e4 DoubleRow PV path (P and V in fp8, 256-key contraction)
- LN rstd via reciprocal_approx_accurate instead of slow DVE reciprocal
- scalar engine reserved for exp in attention; relu/square balanced
  between scalar and vector in dense phases
"""

import numpy as np

import concourse.bass as bass
import concourse.mybir as mybir
import concourse.tile as tile
from concourse.bass_utils import run_bass_kernel_spmd

# ---------------------------------------------------------------- constants
B, S, D, H, DK, DFF = 4, 2048, 256, 8, 32, 1024
NCORES = 8
TOWN = S // 2
QT = 512
NQT = TOWN // QT  # 2
NKC = S // 128  # 16
DC = D // 128  # 2
FC = DFF // 128  # 8
EPS = 1e-5
SCALE = float(1.0 / np.sqrt(np.float32(DK)))
F32 = mybir.dt.float32
BF16 = mybir.dt.bfloat16
FP8 = mybir.dt.float8e4
GROUPS = [[0, 1], [2, 3], [4, 5], [6, 7]]
VW = DK + 1  # V columns per head incl. ones column
PVW = 36  # padded V row pitch (fp8 DoubleRow needs 16B-aligned strides)

PV_FP8 = True  # fp8e4 DoubleRow PV path
EXP_BIAS = -1.0 if PV_FP8 else 0.0  # headroom below fp8e4 max; cancels in softmax


def _legalize_multiwaits(nc):
    """Split multi-wait instructions into prefix EventSemaphore waits."""
    import json

    orig = nc.to_json_bytes

    def patched():
        j = json.loads(orig())
        n = 0
        for fn in j.get("functions", []):
            for bb in fn.get("blocks", []):
                out = []
                for ins in bb.get("instructions", []):
                    si = ins.get("sync_info") or {}
                    waits = si.get("on_wait") or []
                    if len(waits) > 1:
                        for w in waits[:-1]:
                            n += 1
                            out.append({
                                "engine": ins["engine"],
                                "ins": [],
                                "name": f"I-mwsplit-{n}",
                                "opcode": "EventSemaphore",
                                "outs": [],
                                "sync_info": {"on_update": [], "on_wait": [w]},
                            })
                        si["on_wait"] = [waits[-1]]
                    out.append(ins)
                bb["instructions"] = out
        return json.dumps(j).encode()

    nc.to_json_bytes = patched
    return nc


def _mm(nc, out, lhsT, rhs, **kw):
    nc.tensor.matmul(out, lhsT, rhs, **kw)


def _bcast(nc, bc_row, src_row, out_ap):
    """Broadcast a [1, N] SBUF row across partitions via a DRAM bounce."""
    n = src_row.shape[-1]
    row = bc_row[0:n]
    nc.gpsimd.dma_start(row, src_row)
    bcast = bass.AP(tensor=row.tensor, offset=row.offset,
                    ap=[[0, out_ap.shape[0]], *[list(d) for d in row.ap]])
    nc.gpsimd.dma_start(out_ap, bcast)


def build():
    from contextlib import ExitStack

    nc = bass.Bass(num_devices=NCORES)

    xt_in = nc.dram_tensor("xt", [DC, 128, S], BF16, kind="ExternalInput")
    wd = {}
    for bi in range(2):
        for nm in ("wq", "wk", "wv", "wo"):
            wd[f"{nm}{bi}"] = nc.dram_tensor(f"{nm}{bi}", [DC, 128, D], BF16, kind="ExternalInput")
        wd[f"wf1{bi}"] = nc.dram_tensor(f"wf1{bi}", [DC, 128, DFF], BF16, kind="ExternalInput")
        wd[f"wf2{bi}"] = nc.dram_tensor(f"wf2{bi}", [FC, 128, D], BF16, kind="ExternalInput")
        for nm in ("ga", "ba", "gb", "bb"):
            wd[f"{nm}{bi}"] = nc.dram_tensor(f"{nm}{bi}", [DC, 128, 1], F32, kind="ExternalInput")
    out_t = nc.dram_tensor("out_t", [DC, 128, TOWN], F32, kind="ExternalOutput")
    xh_d = nc.dram_tensor("xh_d", [NQT, DC, 128, QT], BF16)
    xg_d = nc.dram_tensor("xg_d", [NQT, 2, DC, 128, QT], BF16)
    bc_d = nc.dram_tensor("bc_d", [16, TOWN], F32)

    with tile.TileContext(nc) as tc, ExitStack() as top:
        top.enter_context(nc.allow_low_precision(
            reason="bf16/fp8 activations; matmul accumulation stays fp32 in PSUM"))
        persist = top.enter_context(tc.tile_pool(name="persist", bufs=1))

        ones32 = persist.tile([128, 32], BF16, tag="ones32", name="ones32")
        nc.vector.memset(ones32, 1.0)
        scale_row = persist.tile([1, 128], BF16, tag="scale_row", name="scale_row")
        nc.vector.memset(scale_row, 1.0 / D)
        eps128 = persist.tile([128, 1], F32, tag="eps128", name="eps128")
        nc.vector.memset(eps128, EPS)
        ebias = persist.tile([128, 1], F32, tag="ebias", name="ebias")
        nc.vector.memset(ebias, EXP_BIAS)
        consts = {"ones32": ones32, "scale_row": scale_row, "eps128": eps128}

        # ---- block input first (unblocks QKV quickly), then weights
        xt = [persist.tile([128, S], BF16, tag=f"xt{i}", name=f"xt{i}") for i in range(DC)]
        for i in range(DC):  # own half first: unblocks the q projection early
            nc.sync.dma_start(xt[i][:, 0:TOWN], xt_in[i][:, 0:TOWN])
        for i in range(DC):
            nc.sync.dma_start(xt[i][:, TOWN:S], xt_in[i][:, TOWN:S])
        W = {}
        for bi in range(2):
            for nm, chunks, width in (
                ("wq", DC, D), ("wk", DC, D), ("wv", DC, D), ("wo", DC, D),
                ("wf1", DC, DFF), ("wf2", FC, D),
            ):
                t = persist.tile([128, chunks, width], BF16, tag=f"{nm}{bi}", name=f"{nm}{bi}")
                for c in range(chunks):
                    nc.sync.dma_start(t[:, c, :], wd[f"{nm}{bi}"][c])
                W[f"{nm}{bi}"] = t
            for nm in ("ga", "ba", "gb", "bb"):
                t = persist.tile([128, DC, 1], F32, tag=f"{nm}{bi}", name=f"{nm}{bi}")
                for c in range(DC):
                    nc.sync.dma_start(t[:, c, :], wd[f"{nm}{bi}"][c])
                W[f"{nm}{bi}"] = t

        # persistent activation tiles (reused across both blocks)
        qT = [persist.tile([128, TOWN], BF16, tag=f"qT{g}", name=f"qT{g}") for g in range(DC)]
        # zero-padded per-head K: only band hh nonzero, so scores are plain
        # full-contraction matmuls (tile_position matmuls slow down adjacent
        # full-array matmuls ~2.6x; measured)
        kTz = [[persist.tile([128, S], BF16, tag=f"kTz{g}_{hh}", name=f"kTz{g}_{hh}")
                for hh in range(4)] for g in range(DC)]
        for g in range(DC):
            for hh in range(4):
                nc.vector.memset(kTz[g][hh], 0.0)
        if PV_FP8:
            vtok = [persist.tile([128, 2, H, PVW], FP8, tag=f"vt{p}", name=f"vt{p}")
                    for p in range(NKC // 2)]
            for p in range(NKC // 2):
                nc.vector.memset(vtok[p][:, :, :, DK:DK + 1], 1.0)
        else:
            vtok = [persist.tile([128, H, VW], BF16, tag=f"vt{k}", name=f"vt{k}")
                    for k in range(NKC)]
            for k in range(NKC):
                nc.vector.memset(vtok[k][:, :, DK:VW], 1.0)
        ot = [persist.tile([128, TOWN], BF16, tag=f"ot{g}", name=f"ot{g}") for g in range(DC)]
        x1 = [persist.tile([128, TOWN], BF16, tag=f"x1_{i}", name=f"x1_{i}") for i in range(DC)]
        hT = persist.tile([128, FC, TOWN], BF16, tag="hT", name="hT")
        x2own = [persist.tile([128, TOWN], BF16, tag=f"x2own{i}", name=f"x2own{i}")
                 for i in range(DC)]
        xout = [persist.tile([128, TOWN], F32, tag=f"xout{i}", name=f"xout{i}")
                for i in range(DC)]
        den = persist.tile([128, 2 * TOWN], F32, tag="den", name="den")
        rb = [persist.tile([128, TOWN], F32, tag=f"rb{g}", name=f"rb{g}") for g in range(DC)]

        for bi in range(2):
            src_q = [xt[i][:, 0:TOWN] for i in range(DC)] if bi == 0 else \
                    [x2own[i][:] for i in range(DC)]
            src_kv = xt
            nc.gpsimd.memset(den[:], 1.0)

            # ============ QKV projections =============================
            with tc.tile_pool(name=f"psA{bi}", bufs=4, space="PSUM") as psA:
                # q^T, own tokens only
                for oc in range(DC):
                    pss = [psA.tile([128, QT], F32, tag="qkv", name="qkv")
                           for _ in range(NQT)]
                    for ic in range(DC):
                        for st_i in range(NQT):
                            _mm(nc, pss[st_i][:], W[f"wq{bi}"][:, ic, oc * 128:(oc + 1) * 128],
                                src_q[ic][:, st_i * QT:(st_i + 1) * QT],
                                start=(ic == 0), stop=(ic == DC - 1))
                    for st_i in range(NQT):
                        nc.vector.tensor_scalar_max(
                            qT[oc][:, st_i * QT:(st_i + 1) * QT], pss[st_i][:], 0.0)
                # k^T over full sequence, written per 32-row head band into
                # the zero-padded kTz tiles
                for oc in range(DC):
                    pss = [psA.tile([128, QT], F32, tag="qkv", name="qkv")
                           for _ in range(S // QT)]
                    for ic in range(DC):
                        for st_i in range(S // QT):
                            _mm(nc, pss[st_i][:], W[f"wk{bi}"][:, ic, oc * 128:(oc + 1) * 128],
                                src_kv[ic][:, st_i * QT:(st_i + 1) * QT],
                                start=(ic == 0), stop=(ic == DC - 1))
                    for st_i in range(S // QT):
                        for hh in range(4):
                            dst = kTz[oc][hh][32 * hh:32 * hh + 32,
                                              st_i * QT:(st_i + 1) * QT]
                            src = pss[st_i][32 * hh:32 * hh + 32, :]
                            if (st_i + hh) % 2 == 0:
                                nc.scalar.activation(dst, src,
                                                     mybir.ActivationFunctionType.Relu)
                            else:
                                nc.vector.tensor_scalar_max(dst, src, 0.0)
                # token-major V with ones column, full sequence
                for kc in range(NKC):
                    ps = psA.tile([128, D], F32, tag="vtok", name="vtok")
                    for ic in range(DC):
                        _mm(nc, ps[:], src_kv[ic][:, kc * 128:(kc + 1) * 128],
                            W[f"wv{bi}"][:, ic, :],
                            start=(ic == 0), stop=(ic == DC - 1))
                    if PV_FP8:
                        dst = vtok[kc // 2][:, kc % 2, :, 0:DK]
                    else:
                        dst = vtok[kc][:, :, 0:DK]
                    src = ps[:].rearrange("p (h k) -> p h k", h=H)
                    if kc % 2 == 0:
                        nc.scalar.activation(dst, src,
                                             mybir.ActivationFunctionType.Relu)
                    else:
                        nc.vector.tensor_scalar_max(dst, src, 0.0)

            # ============ attention ===================================
            with ExitStack() as ast:
                pp = ast.enter_context(tc.tile_pool(name=f"pp{bi}", bufs=3))
                psB = ast.enter_context(tc.tile_pool(name=f"psB{bi}", bufs=2, space="PSUM"))
                psPV = ast.enter_context(tc.tile_pool(name=f"psPV{bi}", bufs=1, space="PSUM"))
                for hp in range(4):  # heads (2hp, 2hp+1)
                    g = hp // 2
                    pv = [psPV.tile([VW, QT], F32, tag=f"pv{x}", name=f"pv{x}")
                          for x in range(4)]  # index 2j+qt
                    if PV_FP8:
                        ptiles = {qt: [] for qt in range(NQT)}
                        for kc in range(NKC):
                            for qt in range(NQT):
                                sc = psB.tile([128, 2, QT], F32, tag="sc", name="sc")
                                for j in range(2):
                                    hh = 2 * (hp % 2) + j
                                    _mm(nc, sc[:, j, :],
                                        kTz[g][hh][:, kc * 128:(kc + 1) * 128],
                                        qT[g][:, qt * QT:(qt + 1) * QT],
                                        start=True, stop=True)
                                if kc % 2 == 0:
                                    ptiles[qt].append(pp.tile(
                                        [128, 2, 2, QT], FP8, tag=f"p4_{qt}", name=f"p4_{qt}"))
                                nc.scalar.activation(
                                    ptiles[qt][-1][:, :, kc % 2, :], sc[:],
                                    mybir.ActivationFunctionType.Exp,
                                    scale=SCALE, bias=ebias[:])
                            if kc % 2 == 0 and kc >= 2:
                                pc = kc // 2 - 1
                                for qt in range(NQT):
                                    for j in range(2):
                                        _mm(nc, pv[2 * j + qt][:],
                                            vtok[pc][:, :, 2 * hp + j, 0:VW],
                                            ptiles[qt][pc][:, j, :, :],
                                            start=(pc == 0), stop=False,
                                            perf_mode=mybir.MatmulPerfMode.DoubleRow,
                                            skip_group_check=True)
                        pc = NKC // 2 - 1
                        for qt in range(NQT):
                            for j in range(2):
                                _mm(nc, pv[2 * j + qt][:],
                                    vtok[pc][:, :, 2 * hp + j, 0:VW],
                                    ptiles[qt][pc][:, j, :, :],
                                    start=False, stop=True,
                                    perf_mode=mybir.MatmulPerfMode.DoubleRow,
                                    skip_group_check=True)
                    else:
                        ptiles = {qt: [] for qt in range(NQT)}
                        for kc in range(NKC):
                            for qt in range(NQT):
                                sc = psB.tile([128, 2, QT], F32, tag="sc", name="sc")
                                for j in range(2):
                                    hh = 2 * (hp % 2) + j
                                    _mm(nc, sc[:, j, :],
                                        kTz[g][hh][:, kc * 128:(kc + 1) * 128],
                                        qT[g][:, qt * QT:(qt + 1) * QT],
                                        start=True, stop=True)
                                p2 = pp.tile([128, 2, QT], BF16, tag=f"p2_{qt}", name=f"p2_{qt}")
                                ptiles[qt].append(p2)
                                nc.scalar.activation(
                                    p2[:], sc[:],
                                    mybir.ActivationFunctionType.Exp,
                                    scale=SCALE, bias=ebias[:])
                            if kc >= 1:
                                for qt in range(NQT):
                                    for j in range(2):
                                        _mm(nc, pv[2 * j + qt][:],
                                            vtok[kc - 1][:, 2 * hp + j, :],
                                            ptiles[qt][kc - 1][:, j, :],
                                            start=(kc - 1 == 0), stop=False,
                                            skip_group_check=True)
                        for qt in range(NQT):
                            for j in range(2):
                                _mm(nc, pv[2 * j + qt][:],
                                    vtok[NKC - 1][:, 2 * hp + j, :],
                                    ptiles[qt][NKC - 1][:, j, :],
                                    start=False, stop=True,
                                    skip_group_check=True)
                    for j in range(2):
                        h = 2 * hp + j
                        hh = 2 * (hp % 2) + j
                        for qt in range(NQT):
                            qsl = slice(qt * QT, (qt + 1) * QT)
                            nc.vector.tensor_copy(
                                ot[g][32 * hh:32 * hh + 32, qsl],
                                pv[2 * j + qt][0:DK, :])
                            nc.vector.tensor_copy(
                                den[32 * (h % 4):32 * (h % 4) + 1,
                                    (h // 4) * TOWN + qt * QT:
                                    (h // 4) * TOWN + (qt + 1) * QT],
                                pv[2 * j + qt][DK:DK + 1, :])
                    # normalize group g as soon as its heads are done so the
                    # tail hides under the remaining attention work. g=0's
                    # reciprocal runs on the (idle-during-attention) DVE;
                    # g=1's on the scalar via exp(-ln(den)) right after the
                    # last exp (ln/exp share one ACT table set).
                    if hp == 1 or hp == 3:
                        g = hp // 2
                        dsl = slice(g * TOWN, (g + 1) * TOWN)
                        if hp == 1:
                            nc.vector.reciprocal(den[:, dsl], den[:, dsl])
                        else:
                            nc.scalar.activation(den[:, dsl], den[:, dsl],
                                                 mybir.ActivationFunctionType.Ln)
                            nc.scalar.activation(den[:, dsl], den[:, dsl],
                                                 mybir.ActivationFunctionType.Exp,
                                                 scale=-1.0)
                        for hh in range(4):
                            h = 4 * g + hh
                            _bcast(nc, bc_d[8 * bi + h],
                                   den[32 * (h % 4):32 * (h % 4) + 1, dsl],
                                   rb[g][32 * hh:32 * hh + 32, :])
                        nc.vector.tensor_mul(ot[g][:], ot[g][:], rb[g][:])

            # ============ Wo proj + residual + LN1 ====================
            self_ln(nc, tc, W, f"ga{bi}", f"ba{bi}", ot, src_q, x1,
                    consts, name=f"ln1_{bi}", wt=W[f"wo{bi}"], nch=DC)

            # ============ FFN + residual + LN2 ========================
            with tc.tile_pool(name=f"psD{bi}", bufs=3, space="PSUM") as psD:
                for fc in range(FC):
                    pss = [psD.tile([128, QT], F32, tag="ffn1", name="ffn1")
                           for _ in range(NQT)]
                    for ic in range(DC):
                        for qt in range(NQT):
                            _mm(nc, pss[qt][:], W[f"wf1{bi}"][:, ic, fc * 128:(fc + 1) * 128],
                                x1[ic][:, qt * QT:(qt + 1) * QT],
                                start=(ic == 0), stop=(ic == DC - 1))
                    for qt in range(NQT):
                        dst = hT[:, fc, qt * QT:(qt + 1) * QT]
                        if (fc + qt) % 2 == 0:
                            nc.scalar.activation(
                                dst, pss[qt][:], mybir.ActivationFunctionType.Relu)
                        else:
                            nc.vector.tensor_scalar_max(dst, pss[qt][:], 0.0)
            hT_moving = [hT[:, fc, :] for fc in range(FC)]
            out_tiles = x2own if bi == 0 else xout
            self_ln(nc, tc, W, f"gb{bi}", f"bb{bi}", hT_moving, x1,
                    out_tiles, consts, name=f"ln2_{bi}", wt=W[f"wf2{bi}"], nch=FC)

            # ============ exchange (after block 0 only) ===============
            if bi == 0:
                for qt in range(NQT):
                    for i in range(DC):
                        nc.sync.dma_start(xh_d[qt, i],
                                          x2own[i][:, qt * QT:(qt + 1) * QT])
                    nc.gpsimd.collective_compute(
                        "AllGather", mybir.AluOpType.bypass,
                        replica_groups=GROUPS,
                        ins=[xh_d[qt].flatten()], outs=[xg_d[qt].flatten()])
                for qt in range(NQT):
                    for r in range(2):
                        for i in range(DC):
                            nc.sync.dma_start(
                                xt[i][:, r * TOWN + qt * QT:r * TOWN + (qt + 1) * QT],
                                xg_d[qt, r, i])

        for i in range(DC):  # chunked so each quarter leaves as it is ready
            for qt in range(NQT):
                qsl = slice(qt * QT, (qt + 1) * QT)
                nc.sync.dma_start(out_t[i][:, qsl], xout[i][:, qsl])

    return _legalize_multiwaits(nc)


def self_ln(nc, tc, W, gkey, bkey, moving, resid, out_tiles, consts, name,
            wt, nch):
    """proj the `moving` chunks with `wt`, relu, add `resid`, layer-norm
    with (gamma=W[gkey], beta=W[bkey]) -> out_tiles."""
    from contextlib import ExitStack

    ones32 = consts["ones32"]
    scale_row = consts["scale_row"]
    with ExitStack() as st:
        tmp = st.enter_context(tc.tile_pool(name=f"{name}_tmp", bufs=1))
        psC = st.enter_context(tc.tile_pool(name=f"{name}_ps", bufs=2, space="PSUM"))
        psS = st.enter_context(tc.tile_pool(name=f"{name}_st", bufs=2, space="PSUM"))
        psB = st.enter_context(tc.tile_pool(name=f"{name}_bc", bufs=2, space="PSUM"))

        y = [tmp.tile([128, TOWN], BF16, tag=f"y{i}", name=f"y{i}") for i in range(DC)]
        srows = tmp.tile([1, 2, NQT, QT], BF16, tag="srows", name="srows")
        mu_b, rstd_b = {}, {}
        for oc in range(DC):
            pss = [psC.tile([128, QT], F32, tag="proj", name="proj")
                   for _ in range(NQT)]
            for ic in range(nch):
                for qt in range(NQT):
                    _mm(nc, pss[qt][:], wt[:, ic, oc * 128:(oc + 1) * 128],
                        moving[ic][:, qt * QT:(qt + 1) * QT],
                        start=(ic == 0), stop=(ic == nch - 1))
            for qt in range(NQT):
                qsl = slice(qt * QT, (qt + 1) * QT)
                # y = relu(ps) + resid
                nc.vector.scalar_tensor_tensor(
                    y[oc][:, qsl], pss[qt][:], 0.0, resid[oc][:, qsl],
                    op0=mybir.AluOpType.max, op1=mybir.AluOpType.add)
        for qt in range(NQT):
            qsl = slice(qt * QT, (qt + 1) * QT)
            sum_ps = psS.tile([32, QT], F32, tag="stat", name="stat")
            sq_ps = psS.tile([32, QT], F32, tag="stat", name="stat")
            for oc in range(DC):
                ysq = tmp.tile([128, QT], BF16, tag="ysq", name="ysq", bufs=3)
                if (oc + qt) % 2 == 0:
                    nc.scalar.activation(ysq[:], y[oc][:, qsl],
                                         mybir.ActivationFunctionType.Square)
                else:
                    nc.vector.tensor_mul(ysq[:], y[oc][:, qsl], y[oc][:, qsl])
                _mm(nc, sum_ps[:], ones32, y[oc][:, qsl],
                    start=(oc == 0), stop=(oc == DC - 1), skip_group_check=True)
                _mm(nc, sq_ps[:], ones32, ysq[:],
                    start=(oc == 0), stop=(oc == DC - 1), skip_group_check=True)
            nc.vector.tensor_copy(srows[:, 0, qt, :], sum_ps[0:1, :])
            nc.vector.tensor_copy(srows[:, 1, qt, :], sq_ps[0:1, :])
            # broadcast mean and mean-square across partitions (K=1 matmuls)
            mb = psB.tile([128, QT], F32, tag="bc", name="bc")
            m2 = psB.tile([128, QT], F32, tag="bc", name="bc")
            _mm(nc, mb[:], scale_row, srows[:, 0, qt, :], start=True, stop=True)
            _mm(nc, m2[:], scale_row, srows[:, 1, qt, :], start=True, stop=True)
            # var = m2 - mu^2 ; rstd = 1/sqrt(var + eps)
            msb = tmp.tile([128, QT], F32, tag="msb", name="msb", bufs=2)
            nc.vector.tensor_copy(msb[:], mb[:])
            vb = tmp.tile([128, QT], F32, tag="vb", name="vb", bufs=2)
            nc.vector.tensor_mul(vb[:], msb[:], msb[:])
            nc.vector.tensor_sub(vb[:], m2[:], vb[:])
            # rstd = exp(-0.5*ln(var+eps)); ln/exp share one ACT table set
            sq = tmp.tile([128, QT], F32, tag="sq", name="sq", bufs=2)
            nc.scalar.activation(sq[:], vb[:],
                                 mybir.ActivationFunctionType.Ln,
                                 bias=consts["eps128"])
            rbt = tmp.tile([128, QT], F32, tag="rb2", name="rb2", bufs=2)
            nc.scalar.activation(rbt[:], sq[:],
                                 mybir.ActivationFunctionType.Exp,
                                 scale=-0.5)
            mu_b[qt], rstd_b[qt] = msb, rbt
        for oc in range(DC):
            for qt in range(NQT):
                qsl = slice(qt * QT, (qt + 1) * QT)
                t = tmp.tile([128, QT], F32, tag="t", name="t", bufs=3)
                nc.vector.tensor_sub(t[:], y[oc][:, qsl], mu_b[qt][:])
                nc.vector.scalar_tensor_tensor(
                    t[:], t[:], W[gkey][:, oc, :], rstd_b[qt][:],
                    op0=mybir.AluOpType.mult, op1=mybir.AluOpType.mult)
                nc.vector.tensor_scalar_add(out_tiles[oc][:, qsl], t[:],
                                            W[bkey][:, oc, :])


def _install_profile_hook():
    import sys as _sys
    import types as _types

    if "antenv.axon_hooks" in _sys.modules:
        return
    _sys.path.insert(0, "/root/.axon_site")
    try:
        from trn_agent_boot.trn_boot import _ntff_profile_via_ctypes
        hook = _ntff_profile_via_ctypes("/opt/axon/libaxon_pjrt.so")
    except Exception:
        hook = None
    mod = _types.ModuleType("antenv.axon_hooks")
    mod.get_axon_ntff_profile_hook = lambda: hook
    mod.set_axon_ntff_profile_hook = lambda h: None
    _sys.modules["antenv.axon_hooks"] = mod


# ---------------------------------------------------------------- host side
_NC_CACHE = {}


def _get_nc(debug=False):
    if debug not in _NC_CACHE:
        _NC_CACHE[debug] = build()
    return _NC_CACHE[debug]


def _prep_inputs(x, weights):
    import ml_dtypes
    bf = ml_dtypes.bfloat16
    in_maps = []
    wmats = {}
    for bi, (q, k, v, o, f1, f2) in enumerate(
        (("W11", "W12", "W13", "W14", "Wf11", "Wf21"),
         ("W21", "W22", "W23", "W24", "Wf12", "Wf22"))):
        wmats[f"wq{bi}"] = np.ascontiguousarray(
            weights[q].T.reshape(DC, 128, D)).astype(bf)
        wmats[f"wk{bi}"] = np.ascontiguousarray(
            weights[k].T.reshape(DC, 128, D)).astype(bf)
        wmats[f"wv{bi}"] = np.ascontiguousarray(
            weights[v].T.reshape(DC, 128, D)).astype(bf)
        wmats[f"wo{bi}"] = np.ascontiguousarray(
            weights[o].T.reshape(DC, 128, D)).astype(bf)
        wmats[f"wf1{bi}"] = np.ascontiguousarray(
            weights[f1].T.reshape(DC, 128, DFF)).astype(bf)
        wmats[f"wf2{bi}"] = np.ascontiguousarray(
            weights[f2].T.reshape(FC, 128, D)).astype(bf)
    for bi, (g1, b1, g2, b2) in enumerate(
        (("g1", "b1", "g2", "b2"), ("g3", "b3", "g4", "b4"))):
        wmats[f"ga{bi}"] = np.ascontiguousarray(weights[g1].reshape(DC, 128, 1))
        wmats[f"ba{bi}"] = np.ascontiguousarray(weights[b1].reshape(DC, 128, 1))
        wmats[f"gb{bi}"] = np.ascontiguousarray(weights[g2].reshape(DC, 128, 1))
        wmats[f"bb{bi}"] = np.ascontiguousarray(weights[b2].reshape(DC, 128, 1))
    for c in range(NCORES):
        b, half = c // 2, c % 2
        xb = x[b]
        own = xb[half * TOWN:(half + 1) * TOWN]
        other = xb[(1 - half) * TOWN:(2 - half) * TOWN]
        xcore = np.concatenate([own, other], axis=0)
        xt = np.ascontiguousarray(xcore.T.reshape(DC, 128, S)).astype(bf)
        m = {"xt": xt}
        m.update(wmats)
        in_maps.append(m)
    return in_maps


def kernel(x, W11, W12, W13, W14, W21, W22, W23, W24,
           Wf11, Wf21, Wf12, Wf22,
           g1, b1, g2, b2, g3, b3, g4, b4, _debug=False, _trace=False):
    weights = dict(W11=W11, W12=W12, W13=W13, W14=W14,
                   W21=W21, W22=W22, W23=W23, W24=W24,
                   Wf11=Wf11, Wf21=Wf21, Wf12=Wf12, Wf22=Wf22,
                   g1=g1, b1=b1, g2=g2, b2=b2, g3=g3, b3=b3, g4=g4, b4=b4)
    weights = {k: np.asarray(v, dtype=np.float32) for k, v in weights.items()}
    x = np.asarray(x, dtype=np.float32)
    if _trace:
        _install_profile_hook()
    nc = _get_nc(_debug)
    in_maps = _prep_inputs(x, weights)
    res = run_bass_kernel_spmd(nc, in_maps, core_ids=list(range(NCORES)),
                               trace=_trace)
    out = np.empty((B, S, D), dtype=np.float32)
    for c in range(NCORES):
        b, half = c // 2, c % 2
        ot = res.results[c]["out_t"].astype(np.float32).reshape(D, TOWN)
        out[b, half * TOWN:(half + 1) * TOWN] = ot.T
    if _debug or _trace:
        kernel.last_result = res
    return out


# revision 15
# speedup vs baseline: 1.1934x; 1.1934x over previous
"""Two-block transformer encoder (B=4, S=2048, D=256, H=8, DFF=1024) on 8
Trainium2 NeuronCores.

Sharding: core c -> batch b = c//2, sequence half = c%2 (1024 tokens owned).
Weights replicated. Block 1 computes K/V over the full sequence and
Q/FFN/LN over the owned half. Between blocks the halves are exchanged
with chunked AllGathers (bf16) that overlap block-2's Q projection.

v2 changes vs baseline:
- everything bf16 (activations, weights, exchange); PSUM stays fp32
- attention loop per (head-quad, qt): 4 score matmuls -> one 2048-wide
  exp on the scalar engine -> 4 PV matmuls; PV emission skewed one kc
  behind scores so the PE never stalls waiting for exp (strict FIFO)
- optional fp8e4 DoubleRow PV path (P and V in fp8, 256-key contraction)
- LN rstd via reciprocal_approx_accurate instead of slow DVE reciprocal
- scalar engine reserved for exp in attention; relu/square balanced
  between scalar and vector in dense phases
"""

import numpy as np

import concourse.bass as bass
import concourse.mybir as mybir
import concourse.tile as tile
from concourse.bass_utils import run_bass_kernel_spmd

# ---------------------------------------------------------------- constants
B, S, D, H, DK, DFF = 4, 2048, 256, 8, 32, 1024
NCORES = 8
TOWN = S // 2
QT = 512
NQT = TOWN // QT  # 2
NKC = S // 128  # 16
DC = D // 128  # 2
FC = DFF // 128  # 8
EPS = 1e-5
SCALE = float(1.0 / np.sqrt(np.float32(DK)))
F32 = mybir.dt.float32
BF16 = mybir.dt.bfloat16
FP8 = mybir.dt.float8e4
GROUPS = [[0, 1], [2, 3], [4, 5], [6, 7]]
VW = DK + 1  # V columns per head incl. ones column
PVW = 36  # padded V row pitch (fp8 DoubleRow needs 16B-aligned strides)

PV_FP8 = True  # fp8e4 DoubleRow PV path
EXP_BIAS = -1.0 if PV_FP8 else 0.0  # headroom below fp8e4 max; cancels in softmax


def _legalize_multiwaits(nc):
    """Split multi-wait instructions into prefix EventSemaphore waits."""
    import json

    orig = nc.to_json_bytes

    def patched():
        j = json.loads(orig())
        n = 0
        for fn in j.get("functions", []):
            for bb in fn.get("blocks", []):
                out = []
                for ins in bb.get("instructions", []):
                    si = ins.get("sync_info") or {}
                    waits = si.get("on_wait") or []
                    if len(waits) > 1:
                        for w in waits[:-1]:
                            n += 1
                            out.append({
                                "engine": ins["engine"],
                                "ins": [],
                                "name": f"I-mwsplit-{n}",
                                "opcode": "EventSemaphore",
                                "outs": [],
                                "sync_info": {"on_update": [], "on_wait": [w]},
                            })
                        si["on_wait"] = [waits[-1]]
                    out.append(ins)
                bb["instructions"] = out
        return json.dumps(j).encode()

    nc.to_json_bytes = patched
    return nc


def _mm(nc, out, lhsT, rhs, **kw):
    nc.tensor.matmul(out, lhsT, rhs, **kw)


def _bcast(nc, bc_row, src_row, out_ap):
    """Broadcast a [1, N] SBUF row across partitions via a DRAM bounce."""
    n = src_row.shape[-1]
    row = bc_row[0:n]
    nc.gpsimd.dma_start(row, src_row)
    bcast = bass.AP(tensor=row.tensor, offset=row.offset,
                    ap=[[0, out_ap.shape[0]], *[list(d) for d in row.ap]])
    nc.gpsimd.dma_start(out_ap, bcast)


def build():
    from contextlib import ExitStack

    nc = bass.Bass(num_devices=NCORES)

    xt_in = nc.dram_tensor("xt", [DC, 128, S], BF16, kind="ExternalInput")
    wd = {}
    for bi in range(2):
        for nm in ("wq", "wk", "wv", "wo"):
            wd[f"{nm}{bi}"] = nc.dram_tensor(f"{nm}{bi}", [DC, 128, D], BF16, kind="ExternalInput")
        wd[f"wf1{bi}"] = nc.dram_tensor(f"wf1{bi}", [DC, 128, DFF], BF16, kind="ExternalInput")
        wd[f"wf2{bi}"] = nc.dram_tensor(f"wf2{bi}", [FC, 128, D], BF16, kind="ExternalInput")
        for nm in ("ga", "ba", "gb", "bb"):
            wd[f"{nm}{bi}"] = nc.dram_tensor(f"{nm}{bi}", [DC, 128, 1], F32, kind="ExternalInput")
    out_t = nc.dram_tensor("out_t", [DC, 128, TOWN], F32, kind="ExternalOutput")
    xh_d = nc.dram_tensor("xh_d", [NQT, DC, 128, QT], BF16)
    xg_d = nc.dram_tensor("xg_d", [NQT, 2, DC, 128, QT], BF16)
    bc_d = nc.dram_tensor("bc_d", [16, TOWN], F32)

    with tile.TileContext(nc) as tc, ExitStack() as top:
        top.enter_context(nc.allow_low_precision(
            reason="bf16/fp8 activations; matmul accumulation stays fp32 in PSUM"))
        persist = top.enter_context(tc.tile_pool(name="persist", bufs=1))

        ones32 = persist.tile([128, 32], BF16, tag="ones32", name="ones32")
        nc.vector.memset(ones32, 1.0)
        scale_row = persist.tile([1, 128], BF16, tag="scale_row", name="scale_row")
        nc.vector.memset(scale_row, 1.0 / D)
        eps128 = persist.tile([128, 1], F32, tag="eps128", name="eps128")
        nc.vector.memset(eps128, EPS)
        ebias = persist.tile([128, 1], F32, tag="ebias", name="ebias")
        nc.vector.memset(ebias, EXP_BIAS)
        consts = {"ones32": ones32, "scale_row": scale_row, "eps128": eps128}

        # ---- block input first (unblocks QKV quickly), then weights
        xt = [persist.tile([128, S], BF16, tag=f"xt{i}", name=f"xt{i}") for i in range(DC)]
        for i in range(DC):  # own half first: unblocks the q projection early
            nc.sync.dma_start(xt[i][:, 0:TOWN], xt_in[i][:, 0:TOWN])
        for i in range(DC):
            nc.sync.dma_start(xt[i][:, TOWN:S], xt_in[i][:, TOWN:S])
        W = {}
        for bi in range(2):
            for nm, chunks, width in (
                ("wq", DC, D), ("wk", DC, D), ("wv", DC, D), ("wo", DC, D),
                ("wf1", DC, DFF), ("wf2", FC, D),
            ):
                t = persist.tile([128, chunks, width], BF16, tag=f"{nm}{bi}", name=f"{nm}{bi}")
                for c in range(chunks):
                    nc.sync.dma_start(t[:, c, :], wd[f"{nm}{bi}"][c])
                W[f"{nm}{bi}"] = t
            for nm in ("ga", "ba", "gb", "bb"):
                t = persist.tile([128, DC, 1], F32, tag=f"{nm}{bi}", name=f"{nm}{bi}")
                for c in range(DC):
                    nc.sync.dma_start(t[:, c, :], wd[f"{nm}{bi}"][c])
                W[f"{nm}{bi}"] = t

        # persistent activation tiles (reused across both blocks)
        qT = [persist.tile([128, TOWN], BF16, tag=f"qT{g}", name=f"qT{g}") for g in range(DC)]
        # zero-padded per-head K: only band hh nonzero, so scores are plain
        # full-contraction matmuls (tile_position matmuls slow down adjacent
        # full-array matmuls ~2.6x; measured)
        kTz = [[persist.tile([128, S], BF16, tag=f"kTz{g}_{hh}", name=f"kTz{g}_{hh}")
                for hh in range(4)] for g in range(DC)]
        for g in range(DC):
            for hh in range(4):
                nc.vector.memset(kTz[g][hh], 0.0)
        if PV_FP8:
            vtok = [persist.tile([128, 2, H, PVW], FP8, tag=f"vt{p}", name=f"vt{p}")
                    for p in range(NKC // 2)]
            for p in range(NKC // 2):
                nc.vector.memset(vtok[p][:, :, :, DK:DK + 1], 1.0)
        else:
            vtok = [persist.tile([128, H, VW], BF16, tag=f"vt{k}", name=f"vt{k}")
                    for k in range(NKC)]
            for k in range(NKC):
                nc.vector.memset(vtok[k][:, :, DK:VW], 1.0)
        ot = [persist.tile([128, TOWN], BF16, tag=f"ot{g}", name=f"ot{g}") for g in range(DC)]
        x1 = [persist.tile([128, TOWN], BF16, tag=f"x1_{i}", name=f"x1_{i}") for i in range(DC)]
        hT = persist.tile([128, FC, TOWN], BF16, tag="hT", name="hT")
        x2own = [persist.tile([128, TOWN], BF16, tag=f"x2own{i}", name=f"x2own{i}")
                 for i in range(DC)]
        xout = [persist.tile([128, TOWN], F32, tag=f"xout{i}", name=f"xout{i}")
                for i in range(DC)]
        den = persist.tile([128, 2 * TOWN], F32, tag="den", name="den")
        rb = [persist.tile([128, TOWN], F32, tag=f"rb{g}", name=f"rb{g}") for g in range(DC)]

        for bi in range(2):
            src_q = [xt[i][:, 0:TOWN] for i in range(DC)] if bi == 0 else \
                    [x2own[i][:] for i in range(DC)]
            src_kv = xt
            nc.gpsimd.memset(den[:], 1.0)

            # ============ QKV projections =============================
            with tc.tile_pool(name=f"psA{bi}", bufs=4, space="PSUM") as psA:
                # q^T, own tokens only
                for oc in range(DC):
                    pss = [psA.tile([128, QT], F32, tag="qkv", name="qkv")
                           for _ in range(NQT)]
                    for ic in range(DC):
                        for st_i in range(NQT):
                            _mm(nc, pss[st_i][:], W[f"wq{bi}"][:, ic, oc * 128:(oc + 1) * 128],
                                src_q[ic][:, st_i * QT:(st_i + 1) * QT],
                                start=(ic == 0), stop=(ic == DC - 1))
                    for st_i in range(NQT):
                        nc.vector.tensor_scalar_max(
                            qT[oc][:, st_i * QT:(st_i + 1) * QT], pss[st_i][:], 0.0)
                # k^T over full sequence, written per 32-row head band into
                # the zero-padded kTz tiles
                for oc in range(DC):
                    pss = [psA.tile([128, QT], F32, tag="qkv", name="qkv")
                           for _ in range(S // QT)]
                    for ic in range(DC):
                        for st_i in range(S // QT):
                            _mm(nc, pss[st_i][:], W[f"wk{bi}"][:, ic, oc * 128:(oc + 1) * 128],
                                src_kv[ic][:, st_i * QT:(st_i + 1) * QT],
                                start=(ic == 0), stop=(ic == DC - 1))
                    for st_i in range(S // QT):
                        for hh in range(4):
                            dst = kTz[oc][hh][32 * hh:32 * hh + 32,
                                              st_i * QT:(st_i + 1) * QT]
                            src = pss[st_i][32 * hh:32 * hh + 32, :]
                            if (st_i + hh) % 2 == 0:
                                nc.scalar.activation(dst, src,
                                                     mybir.ActivationFunctionType.Relu)
                            else:
                                nc.vector.tensor_scalar_max(dst, src, 0.0)
                # token-major V with ones column, full sequence
                for kc in range(NKC):
                    ps = psA.tile([128, D], F32, tag="vtok", name="vtok")
                    for ic in range(DC):
                        _mm(nc, ps[:], src_kv[ic][:, kc * 128:(kc + 1) * 128],
                            W[f"wv{bi}"][:, ic, :],
                            start=(ic == 0), stop=(ic == DC - 1))
                    if PV_FP8:
                        dst = vtok[kc // 2][:, kc % 2, :, 0:DK]
                    else:
                        dst = vtok[kc][:, :, 0:DK]
                    src = ps[:].rearrange("p (h k) -> p h k", h=H)
                    if kc % 2 == 0:
                        nc.scalar.activation(dst, src,
                                             mybir.ActivationFunctionType.Relu)
                    else:
                        nc.vector.tensor_scalar_max(dst, src, 0.0)

            # ============ attention ===================================
            with ExitStack() as ast:
                pp = ast.enter_context(tc.tile_pool(name=f"pp{bi}", bufs=3))
                psB = ast.enter_context(tc.tile_pool(name=f"psB{bi}", bufs=2, space="PSUM"))
                psPV = ast.enter_context(tc.tile_pool(name=f"psPV{bi}", bufs=1, space="PSUM"))
                for hp in range(4):  # heads (2hp, 2hp+1)
                    g = hp // 2
                    pv = [psPV.tile([VW, QT], F32, tag=f"pv{x}", name=f"pv{x}")
                          for x in range(4)]  # index 2j+qt
                    if PV_FP8:
                        ptiles = {qt: [] for qt in range(NQT)}
                        for kc in range(NKC):
                            # j outer / qt inner: consecutive matmuls share the
                            # stationary operand (LDWEIGHTS is not overlapped
                            # by this compiler, but repeats are skipped)
                            scs = [psB.tile([128, 2, QT], F32, tag="sc", name="sc")
                                   for _ in range(NQT)]
                            for j in range(2):
                                hh = 2 * (hp % 2) + j
                                for qt in range(NQT):
                                    _mm(nc, scs[qt][:, j, :],
                                        kTz[g][hh][:, kc * 128:(kc + 1) * 128],
                                        qT[g][:, qt * QT:(qt + 1) * QT],
                                        start=True, stop=True)
                                    if j == 1:
                                        if kc % 2 == 0:
                                            ptiles[qt].append(pp.tile(
                                                [128, 2, 2, QT], FP8,
                                                tag=f"p4_{qt}", name=f"p4_{qt}"))
                                        nc.scalar.activation(
                                            ptiles[qt][-1][:, :, kc % 2, :],
                                            scs[qt][:],
                                            mybir.ActivationFunctionType.Exp,
                                            scale=SCALE, bias=ebias[:])
                            if kc % 2 == 0 and kc >= 2:
                                pc = kc // 2 - 1
                                for qt in range(NQT):
                                    for j in range(2):
                                        _mm(nc, pv[2 * j + qt][:],
                                            vtok[pc][:, :, 2 * hp + j, 0:VW],
                                            ptiles[qt][pc][:, j, :, :],
                                            start=(pc == 0), stop=False,
                                            perf_mode=mybir.MatmulPerfMode.DoubleRow,
                                            skip_group_check=True)
                        pc = NKC // 2 - 1
                        for qt in range(NQT):
                            for j in range(2):
                                _mm(nc, pv[2 * j + qt][:],
                                    vtok[pc][:, :, 2 * hp + j, 0:VW],
                                    ptiles[qt][pc][:, j, :, :],
                                    start=False, stop=True,
                                    perf_mode=mybir.MatmulPerfMode.DoubleRow,
                                    skip_group_check=True)
                    else:
                        ptiles = {qt: [] for qt in range(NQT)}
                        for kc in range(NKC):
                            for qt in range(NQT):
                                sc = psB.tile([128, 2, QT], F32, tag="sc", name="sc")
                                for j in range(2):
                                    hh = 2 * (hp % 2) + j
                                    _mm(nc, sc[:, j, :],
                                        kTz[g][hh][:, kc * 128:(kc + 1) * 128],
                                        qT[g][:, qt * QT:(qt + 1) * QT],
                                        start=True, stop=True)
                                p2 = pp.tile([128, 2, QT], BF16, tag=f"p2_{qt}", name=f"p2_{qt}")
                                ptiles[qt].append(p2)
                                nc.scalar.activation(
                                    p2[:], sc[:],
                                    mybir.ActivationFunctionType.Exp,
                                    scale=SCALE, bias=ebias[:])
                            if kc >= 1:
                                for qt in range(NQT):
                                    for j in range(2):
                                        _mm(nc, pv[2 * j + qt][:],
                                            vtok[kc - 1][:, 2 * hp + j, :],
                                            ptiles[qt][kc - 1][:, j, :],
                                            start=(kc - 1 == 0), stop=False,
                                            skip_group_check=True)
                        for qt in range(NQT):
                            for j in range(2):
                                _mm(nc, pv[2 * j + qt][:],
                                    vtok[NKC - 1][:, 2 * hp + j, :],
                                    ptiles[qt][NKC - 1][:, j, :],
                                    start=False, stop=True,
                                    skip_group_check=True)
                    for j in range(2):
                        h = 2 * hp + j
                        hh = 2 * (hp % 2) + j
                        for qt in range(NQT):
                            qsl = slice(qt * QT, (qt + 1) * QT)
                            nc.vector.tensor_copy(
                                ot[g][32 * hh:32 * hh + 32, qsl],
                                pv[2 * j + qt][0:DK, :])
                            nc.vector.tensor_copy(
                                den[32 * (h % 4):32 * (h % 4) + 1,
                                    (h // 4) * TOWN + qt * QT:
                                    (h // 4) * TOWN + (qt + 1) * QT],
                                pv[2 * j + qt][DK:DK + 1, :])
                    # normalize group g as soon as its heads are done so the
                    # tail hides under the remaining attention work. g=0's
                    # reciprocal runs on the (idle-during-attention) DVE;
                    # g=1's on the scalar via exp(-ln(den)) right after the
                    # last exp (ln/exp share one ACT table set).
                    if hp == 1 or hp == 3:
                        g = hp // 2
                        dsl = slice(g * TOWN, (g + 1) * TOWN)
                        if hp == 1:
                            nc.vector.reciprocal(den[:, dsl], den[:, dsl])
                        else:
                            nc.scalar.activation(den[:, dsl], den[:, dsl],
                                                 mybir.ActivationFunctionType.Ln)
                            nc.scalar.activation(den[:, dsl], den[:, dsl],
                                                 mybir.ActivationFunctionType.Exp,
                                                 scale=-1.0)
                        for hh in range(4):
                            h = 4 * g + hh
                            _bcast(nc, bc_d[8 * bi + h],
                                   den[32 * (h % 4):32 * (h % 4) + 1, dsl],
                                   rb[g][32 * hh:32 * hh + 32, :])
                        nc.vector.tensor_mul(ot[g][:], ot[g][:], rb[g][:])

            # ============ Wo proj + residual + LN1 ====================
            self_ln(nc, tc, W, f"ga{bi}", f"ba{bi}", ot, src_q, x1,
                    consts, name=f"ln1_{bi}", wt=W[f"wo{bi}"], nch=DC)

            # ============ FFN + residual + LN2 ========================
            with tc.tile_pool(name=f"psD{bi}", bufs=3, space="PSUM") as psD:
                for fc in range(FC):
                    pss = [psD.tile([128, QT], F32, tag="ffn1", name="ffn1")
                           for _ in range(NQT)]
                    for ic in range(DC):
                        for qt in range(NQT):
                            _mm(nc, pss[qt][:], W[f"wf1{bi}"][:, ic, fc * 128:(fc + 1) * 128],
                                x1[ic][:, qt * QT:(qt + 1) * QT],
                                start=(ic == 0), stop=(ic == DC - 1))
                    for qt in range(NQT):
                        dst = hT[:, fc, qt * QT:(qt + 1) * QT]
                        if (fc + qt) % 2 == 0:
                            nc.scalar.activation(
                                dst, pss[qt][:], mybir.ActivationFunctionType.Relu)
                        else:
                            nc.vector.tensor_scalar_max(dst, pss[qt][:], 0.0)
            hT_moving = [hT[:, fc, :] for fc in range(FC)]
            out_tiles = x2own if bi == 0 else xout
            self_ln(nc, tc, W, f"gb{bi}", f"bb{bi}", hT_moving, x1,
                    out_tiles, consts, name=f"ln2_{bi}", wt=W[f"wf2{bi}"], nch=FC)

            # ============ exchange (after block 0 only) ===============
            if bi == 0:
                for qt in range(NQT):
                    for i in range(DC):
                        nc.sync.dma_start(xh_d[qt, i],
                                          x2own[i][:, qt * QT:(qt + 1) * QT])
                    nc.gpsimd.collective_compute(
                        "AllGather", mybir.AluOpType.bypass,
                        replica_groups=GROUPS,
                        ins=[xh_d[qt].flatten()], outs=[xg_d[qt].flatten()])
                for qt in range(NQT):
                    for r in range(2):
                        for i in range(DC):
                            nc.sync.dma_start(
                                xt[i][:, r * TOWN + qt * QT:r * TOWN + (qt + 1) * QT],
                                xg_d[qt, r, i])

        for i in range(DC):  # chunked so each quarter leaves as it is ready
            for qt in range(NQT):
                qsl = slice(qt * QT, (qt + 1) * QT)
                nc.sync.dma_start(out_t[i][:, qsl], xout[i][:, qsl])

    return _legalize_multiwaits(nc)


def self_ln(nc, tc, W, gkey, bkey, moving, resid, out_tiles, consts, name,
            wt, nch):
    """proj the `moving` chunks with `wt`, relu, add `resid`, layer-norm
    with (gamma=W[gkey], beta=W[bkey]) -> out_tiles."""
    from contextlib import ExitStack

    ones32 = consts["ones32"]
    scale_row = consts["scale_row"]
    with ExitStack() as st:
        tmp = st.enter_context(tc.tile_pool(name=f"{name}_tmp", bufs=1))
        psC = st.enter_context(tc.tile_pool(name=f"{name}_ps", bufs=2, space="PSUM"))
        psS = st.enter_context(tc.tile_pool(name=f"{name}_st", bufs=2, space="PSUM"))
        psB = st.enter_context(tc.tile_pool(name=f"{name}_bc", bufs=2, space="PSUM"))

        y = [tmp.tile([128, TOWN], BF16, tag=f"y{i}", name=f"y{i}") for i in range(DC)]
        srows = tmp.tile([1, 2, NQT, QT], BF16, tag="srows", name="srows")
        mu_b, rstd_b = {}, {}
        for oc in range(DC):
            pss = [psC.tile([128, QT], F32, tag="proj", name="proj")
                   for _ in range(NQT)]
            for ic in range(nch):
                for qt in range(NQT):
                    _mm(nc, pss[qt][:], wt[:, ic, oc * 128:(oc + 1) * 128],
                        moving[ic][:, qt * QT:(qt + 1) * QT],
                        start=(ic == 0), stop=(ic == nch - 1))
            for qt in range(NQT):
                qsl = slice(qt * QT, (qt + 1) * QT)
                # y = relu(ps) + resid
                nc.vector.scalar_tensor_tensor(
                    y[oc][:, qsl], pss[qt][:], 0.0, resid[oc][:, qsl],
                    op0=mybir.AluOpType.max, op1=mybir.AluOpType.add)
        for qt in range(NQT):
            qsl = slice(qt * QT, (qt + 1) * QT)
            sum_ps = psS.tile([32, QT], F32, tag="stat", name="stat")
            sq_ps = psS.tile([32, QT], F32, tag="stat", name="stat")
            for oc in range(DC):
                ysq = tmp.tile([128, QT], BF16, tag="ysq", name="ysq", bufs=3)
                if (oc + qt) % 2 == 0:
                    nc.scalar.activation(ysq[:], y[oc][:, qsl],
                                         mybir.ActivationFunctionType.Square)
                else:
                    nc.vector.tensor_mul(ysq[:], y[oc][:, qsl], y[oc][:, qsl])
                _mm(nc, sum_ps[:], ones32, y[oc][:, qsl],
                    start=(oc == 0), stop=(oc == DC - 1), skip_group_check=True)
                _mm(nc, sq_ps[:], ones32, ysq[:],
                    start=(oc == 0), stop=(oc == DC - 1), skip_group_check=True)
            nc.vector.tensor_copy(srows[:, 0, qt, :], sum_ps[0:1, :])
            nc.vector.tensor_copy(srows[:, 1, qt, :], sq_ps[0:1, :])
            # broadcast mean and mean-square across partitions (K=1 matmuls)
            mb = psB.tile([128, QT], F32, tag="bc", name="bc")
            m2 = psB.tile([128, QT], F32, tag="bc", name="bc")
            _mm(nc, mb[:], scale_row, srows[:, 0, qt, :], start=True, stop=True)
            _mm(nc, m2[:], scale_row, srows[:, 1, qt, :], start=True, stop=True)
            # var = m2 - mu^2 ; rstd = 1/sqrt(var + eps)
            msb = tmp.tile([128, QT], F32, tag="msb", name="msb", bufs=2)
            nc.vector.tensor_copy(msb[:], mb[:])
            vb = tmp.tile([128, QT], F32, tag="vb", name="vb", bufs=2)
            nc.vector.tensor_mul(vb[:], msb[:], msb[:])
            nc.vector.tensor_sub(vb[:], m2[:], vb[:])
            # rstd = exp(-0.5*ln(var+eps)); ln/exp share one ACT table set
            sq = tmp.tile([128, QT], F32, tag="sq", name="sq", bufs=2)
            nc.scalar.activation(sq[:], vb[:],
                                 mybir.ActivationFunctionType.Ln,
                                 bias=consts["eps128"])
            rbt = tmp.tile([128, QT], F32, tag="rb2", name="rb2", bufs=2)
            nc.scalar.activation(rbt[:], sq[:],
                                 mybir.ActivationFunctionType.Exp,
                                 scale=-0.5)
            mu_b[qt], rstd_b[qt] = msb, rbt
        for oc in range(DC):
            for qt in range(NQT):
                qsl = slice(qt * QT, (qt + 1) * QT)
                t = tmp.tile([128, QT], F32, tag="t", name="t", bufs=3)
                nc.vector.tensor_sub(t[:], y[oc][:, qsl], mu_b[qt][:])
                nc.vector.scalar_tensor_tensor(
                    t[:], t[:], W[gkey][:, oc, :], rstd_b[qt][:],
                    op0=mybir.AluOpType.mult, op1=mybir.AluOpType.mult)
                nc.vector.tensor_scalar_add(out_tiles[oc][:, qsl], t[:],
                                            W[bkey][:, oc, :])


def _install_profile_hook():
    import sys as _sys
    import types as _types

    if "antenv.axon_hooks" in _sys.modules:
        return
    _sys.path.insert(0, "/root/.axon_site")
    try:
        from trn_agent_boot.trn_boot import _ntff_profile_via_ctypes
        hook = _ntff_profile_via_ctypes("/opt/axon/libaxon_pjrt.so")
    except Exception:
        hook = None
    mod = _types.ModuleType("antenv.axon_hooks")
    mod.get_axon_ntff_profile_hook = lambda: hook
    mod.set_axon_ntff_profile_hook = lambda h: None
    _sys.modules["antenv.axon_hooks"] = mod


# ---------------------------------------------------------------- host side
_NC_CACHE = {}


def _get_nc(debug=False):
    if debug not in _NC_CACHE:
        _NC_CACHE[debug] = build()
    return _NC_CACHE[debug]


def _prep_inputs(x, weights):
    import ml_dtypes
    bf = ml_dtypes.bfloat16
    in_maps = []
    wmats = {}
    for bi, (q, k, v, o, f1, f2) in enumerate(
        (("W11", "W12", "W13", "W14", "Wf11", "Wf21"),
         ("W21", "W22", "W23", "W24", "Wf12", "Wf22"))):
        wmats[f"wq{bi}"] = np.ascontiguousarray(
            weights[q].T.reshape(DC, 128, D)).astype(bf)
        wmats[f"wk{bi}"] = np.ascontiguousarray(
            weights[k].T.reshape(DC, 128, D)).astype(bf)
        wmats[f"wv{bi}"] = np.ascontiguousarray(
            weights[v].T.reshape(DC, 128, D)).astype(bf)
        wmats[f"wo{bi}"] = np.ascontiguousarray(
            weights[o].T.reshape(DC, 128, D)).astype(bf)
        wmats[f"wf1{bi}"] = np.ascontiguousarray(
            weights[f1].T.reshape(DC, 128, DFF)).astype(bf)
        wmats[f"wf2{bi}"] = np.ascontiguousarray(
            weights[f2].T.reshape(FC, 128, D)).astype(bf)
    for bi, (g1, b1, g2, b2) in enumerate(
        (("g1", "b1", "g2", "b2"), ("g3", "b3", "g4", "b4"))):
        wmats[f"ga{bi}"] = np.ascontiguousarray(weights[g1].reshape(DC, 128, 1))
        wmats[f"ba{bi}"] = np.ascontiguousarray(weights[b1].reshape(DC, 128, 1))
        wmats[f"gb{bi}"] = np.ascontiguousarray(weights[g2].reshape(DC, 128, 1))
        wmats[f"bb{bi}"] = np.ascontiguousarray(weights[b2].reshape(DC, 128, 1))
    for c in range(NCORES):
        b, half = c // 2, c % 2
        xb = x[b]
        own = xb[half * TOWN:(half + 1) * TOWN]
        other = xb[(1 - half) * TOWN:(2 - half) * TOWN]
        xcore = np.concatenate([own, other], axis=0)
        xt = np.ascontiguousarray(xcore.T.reshape(DC, 128, S)).astype(bf)
        m = {"xt": xt}
        m.update(wmats)
        in_maps.append(m)
    return in_maps


def kernel(x, W11, W12, W13, W14, W21, W22, W23, W24,
           Wf11, Wf21, Wf12, Wf22,
           g1, b1, g2, b2, g3, b3, g4, b4, _debug=False, _trace=False):
    weights = dict(W11=W11, W12=W12, W13=W13, W14=W14,
                   W21=W21, W22=W22, W23=W23, W24=W24,
                   Wf11=Wf11, Wf21=Wf21, Wf12=Wf12, Wf22=Wf22,
                   g1=g1, b1=b1, g2=g2, b2=b2, g3=g3, b3=b3, g4=g4, b4=b4)
    weights = {k: np.asarray(v, dtype=np.float32) for k, v in weights.items()}
    x = np.asarray(x, dtype=np.float32)
    if _trace:
        _install_profile_hook()
    nc = _get_nc(_debug)
    in_maps = _prep_inputs(x, weights)
    res = run_bass_kernel_spmd(nc, in_maps, core_ids=list(range(NCORES)),
                               trace=_trace)
    out = np.empty((B, S, D), dtype=np.float32)
    for c in range(NCORES):
        b, half = c // 2, c % 2
        ot = res.results[c]["out_t"].astype(np.float32).reshape(D, TOWN)
        out[b, half * TOWN:(half + 1) * TOWN] = ot.T
    if _debug or _trace:
        kernel.last_result = res
    return out


# revision 16
# speedup vs baseline: 1.2482x; 1.0459x over previous
"""Two-block transformer encoder (B=4, S=2048, D=256, H=8, DFF=1024) on 8
Trainium2 NeuronCores.

Sharding: core c -> batch b = c//2, sequence half = c%2 (1024 tokens owned).
Weights replicated. Block 1 computes K/V over the full sequence and
Q/FFN/LN over the owned half. Between blocks the halves are exchanged
with chunked AllGathers (bf16) that overlap block-2's Q projection.

v2 changes vs baseline:
- everything bf16 (activations, weights, exchange); PSUM stays fp32
- attention loop per (head-quad, qt): 4 score matmuls -> one 2048-wide
  exp on the scalar engine -> 4 PV matmuls; PV emission skewed one kc
  behind scores so the PE never stalls waiting for exp (strict FIFO)
- optional fp8e4 DoubleRow PV path (P and V in fp8, 256-key contraction)
- LN rstd via reciprocal_approx_accurate instead of slow DVE reciprocal
- scalar engine reserved for exp in attention; relu/square balanced
  between scalar and vector in dense phases
"""

import numpy as np

import concourse.bass as bass
import concourse.mybir as mybir
import concourse.tile as tile
from concourse.bass_utils import run_bass_kernel_spmd

# ---------------------------------------------------------------- constants
B, S, D, H, DK, DFF = 4, 2048, 256, 8, 32, 1024
NCORES = 8
TOWN = S // 2
QT = 512
NQT = TOWN // QT  # 2
NKC = S // 128  # 16
DC = D // 128  # 2
FC = DFF // 128  # 8
EPS = 1e-5
SCALE = float(1.0 / np.sqrt(np.float32(DK)))
F32 = mybir.dt.float32
BF16 = mybir.dt.bfloat16
FP8 = mybir.dt.float8e4
GROUPS = [[0, 1], [2, 3], [4, 5], [6, 7]]
VW = DK + 1  # V columns per head incl. ones column
PVW = 36  # padded V row pitch (fp8 DoubleRow needs 16B-aligned strides)

PV_FP8 = True  # fp8e4 DoubleRow PV path
EXP_BIAS = -1.0 if PV_FP8 else 0.0  # headroom below fp8e4 max; cancels in softmax


def _legalize_multiwaits(nc):
    """Split multi-wait instructions into prefix EventSemaphore waits."""
    import json

    orig = nc.to_json_bytes

    def patched():
        j = json.loads(orig())
        n = 0
        for fn in j.get("functions", []):
            for bb in fn.get("blocks", []):
                out = []
                for ins in bb.get("instructions", []):
                    si = ins.get("sync_info") or {}
                    waits = si.get("on_wait") or []
                    if len(waits) > 1:
                        for w in waits[:-1]:
                            n += 1
                            out.append({
                                "engine": ins["engine"],
                                "ins": [],
                                "name": f"I-mwsplit-{n}",
                                "opcode": "EventSemaphore",
                                "outs": [],
                                "sync_info": {"on_update": [], "on_wait": [w]},
                            })
                        si["on_wait"] = [waits[-1]]
                    out.append(ins)
                bb["instructions"] = out
        return json.dumps(j).encode()

    nc.to_json_bytes = patched
    return nc


def _mm(nc, out, lhsT, rhs, **kw):
    nc.tensor.matmul(out, lhsT, rhs, **kw)


def _bcast(nc, bc_row, src_row, out_ap):
    """Broadcast a [1, N] SBUF row across partitions via a DRAM bounce."""
    n = src_row.shape[-1]
    row = bc_row[0:n]
    nc.gpsimd.dma_start(row, src_row)
    bcast = bass.AP(tensor=row.tensor, offset=row.offset,
                    ap=[[0, out_ap.shape[0]], *[list(d) for d in row.ap]])
    nc.gpsimd.dma_start(out_ap, bcast)


def build():
    from contextlib import ExitStack

    nc = bass.Bass(num_devices=NCORES)

    xt_in = nc.dram_tensor("xt", [DC, 128, S], BF16, kind="ExternalInput")
    wd = {}
    for bi in range(2):
        for nm in ("wq", "wk", "wv", "wo"):
            wd[f"{nm}{bi}"] = nc.dram_tensor(f"{nm}{bi}", [DC, 128, D], BF16, kind="ExternalInput")
        wd[f"wf1{bi}"] = nc.dram_tensor(f"wf1{bi}", [DC, 128, DFF], BF16, kind="ExternalInput")
        wd[f"wf2{bi}"] = nc.dram_tensor(f"wf2{bi}", [FC, 128, D], BF16, kind="ExternalInput")
        for nm in ("ga", "ba", "gb", "bb"):
            wd[f"{nm}{bi}"] = nc.dram_tensor(f"{nm}{bi}", [DC, 128, 1], F32, kind="ExternalInput")
    out_t = nc.dram_tensor("out_t", [DC, 128, TOWN], F32, kind="ExternalOutput")
    xh_d = nc.dram_tensor("xh_d", [NQT, DC, 128, QT], BF16)
    xg_d = nc.dram_tensor("xg_d", [NQT, 2, DC, 128, QT], BF16)
    bc_d = nc.dram_tensor("bc_d", [16, TOWN], F32)

    with tile.TileContext(nc) as tc, ExitStack() as top:
        top.enter_context(nc.allow_low_precision(
            reason="bf16/fp8 activations; matmul accumulation stays fp32 in PSUM"))
        persist = top.enter_context(tc.tile_pool(name="persist", bufs=1))

        ones32 = persist.tile([128, 32], BF16, tag="ones32", name="ones32")
        nc.vector.memset(ones32, 1.0)
        scale_row = persist.tile([1, 128], BF16, tag="scale_row", name="scale_row")
        nc.vector.memset(scale_row, 1.0 / D)
        eps128 = persist.tile([128, 1], F32, tag="eps128", name="eps128")
        nc.vector.memset(eps128, EPS)
        ebias = persist.tile([128, 1], F32, tag="ebias", name="ebias")
        nc.vector.memset(ebias, EXP_BIAS)
        consts = {"ones32": ones32, "scale_row": scale_row, "eps128": eps128}

        # ---- block input first (unblocks QKV quickly), then weights
        xt = [persist.tile([128, S], BF16, tag=f"xt{i}", name=f"xt{i}") for i in range(DC)]
        for i in range(DC):  # own half first: unblocks the q projection early
            nc.sync.dma_start(xt[i][:, 0:TOWN], xt_in[i][:, 0:TOWN])
        for i in range(DC):
            nc.sync.dma_start(xt[i][:, TOWN:S], xt_in[i][:, TOWN:S])
        W = {}
        for bi in range(2):
            for nm, chunks, width in (
                ("wq", DC, D), ("wk", DC, D), ("wv", DC, D), ("wo", DC, D),
                ("wf1", DC, DFF), ("wf2", FC, D),
            ):
                t = persist.tile([128, chunks, width], BF16, tag=f"{nm}{bi}", name=f"{nm}{bi}")
                for c in range(chunks):
                    nc.sync.dma_start(t[:, c, :], wd[f"{nm}{bi}"][c])
                W[f"{nm}{bi}"] = t
            for nm in ("ga", "ba", "gb", "bb"):
                t = persist.tile([128, DC, 1], F32, tag=f"{nm}{bi}", name=f"{nm}{bi}")
                for c in range(DC):
                    nc.sync.dma_start(t[:, c, :], wd[f"{nm}{bi}"][c])
                W[f"{nm}{bi}"] = t

        # persistent activation tiles (reused across both blocks)
        # zero-padded per-head Q (moving side): only band hh nonzero, so
        # scores are plain full-contraction matmuls (tile_position matmuls
        # slow down adjacent full-array matmuls ~2.6x; measured) and all four
        # (j, qt) score matmuls of a kc share one packed-K stationary
        # (LDWEIGHTS is not overlapped by this compiler; repeats are skipped).
        qTz = [[persist.tile([128, TOWN], BF16, tag=f"qTz{g}_{hh}", name=f"qTz{g}_{hh}")
                for hh in range(4)] for g in range(DC)]
        for g in range(DC):
            for hh in range(4):
                nc.vector.memset(qTz[g][hh], 0.0)
        kT = [persist.tile([128, S], BF16, tag=f"kT{g}", name=f"kT{g}") for g in range(DC)]
        if PV_FP8:
            vtok = [persist.tile([128, 2, H, PVW], FP8, tag=f"vt{p}", name=f"vt{p}")
                    for p in range(NKC // 2)]
            for p in range(NKC // 2):
                nc.vector.memset(vtok[p][:, :, :, DK:DK + 1], 1.0)
        else:
            vtok = [persist.tile([128, H, VW], BF16, tag=f"vt{k}", name=f"vt{k}")
                    for k in range(NKC)]
            for k in range(NKC):
                nc.vector.memset(vtok[k][:, :, DK:VW], 1.0)
        ot = [persist.tile([128, TOWN], BF16, tag=f"ot{g}", name=f"ot{g}") for g in range(DC)]
        x1 = [persist.tile([128, TOWN], BF16, tag=f"x1_{i}", name=f"x1_{i}") for i in range(DC)]
        hT = persist.tile([128, FC, TOWN], BF16, tag="hT", name="hT")
        x2own = [persist.tile([128, TOWN], BF16, tag=f"x2own{i}", name=f"x2own{i}")
                 for i in range(DC)]
        xout = [persist.tile([128, TOWN], F32, tag=f"xout{i}", name=f"xout{i}")
                for i in range(DC)]
        den = persist.tile([128, 2 * TOWN], F32, tag="den", name="den")
        rb = [persist.tile([128, TOWN], F32, tag=f"rb{g}", name=f"rb{g}") for g in range(DC)]

        for bi in range(2):
            src_q = [xt[i][:, 0:TOWN] for i in range(DC)] if bi == 0 else \
                    [x2own[i][:] for i in range(DC)]
            src_kv = xt
            nc.gpsimd.memset(den[:], 1.0)

            # ============ QKV projections =============================
            with tc.tile_pool(name=f"psA{bi}", bufs=4, space="PSUM") as psA:
                # q^T, own tokens only
                for oc in range(DC):
                    pss = [psA.tile([128, QT], F32, tag="qkv", name="qkv")
                           for _ in range(NQT)]
                    for ic in range(DC):
                        for st_i in range(NQT):
                            _mm(nc, pss[st_i][:], W[f"wq{bi}"][:, ic, oc * 128:(oc + 1) * 128],
                                src_q[ic][:, st_i * QT:(st_i + 1) * QT],
                                start=(ic == 0), stop=(ic == DC - 1))
                    for st_i in range(NQT):
                        for hh in range(4):
                            dst = qTz[oc][hh][32 * hh:32 * hh + 32,
                                              st_i * QT:(st_i + 1) * QT]
                            src = pss[st_i][32 * hh:32 * hh + 32, :]
                            if (st_i + hh) % 2 == 0:
                                nc.scalar.activation(dst, src,
                                                     mybir.ActivationFunctionType.Relu)
                            else:
                                nc.vector.tensor_scalar_max(dst, src, 0.0)
                # k^T over full sequence (packed; heads on 32-row bands)
                for oc in range(DC):
                    pss = [psA.tile([128, QT], F32, tag="qkv", name="qkv")
                           for _ in range(S // QT)]
                    for ic in range(DC):
                        for st_i in range(S // QT):
                            _mm(nc, pss[st_i][:], W[f"wk{bi}"][:, ic, oc * 128:(oc + 1) * 128],
                                src_kv[ic][:, st_i * QT:(st_i + 1) * QT],
                                start=(ic == 0), stop=(ic == DC - 1))
                    for st_i in range(S // QT):
                        dst = kT[oc][:, st_i * QT:(st_i + 1) * QT]
                        if st_i % 2 == 0:
                            nc.scalar.activation(dst, pss[st_i][:],
                                                 mybir.ActivationFunctionType.Relu)
                        else:
                            nc.vector.tensor_scalar_max(dst, pss[st_i][:], 0.0)
                # token-major V with ones column, full sequence
                for kc in range(NKC):
                    ps = psA.tile([128, D], F32, tag="vtok", name="vtok")
                    for ic in range(DC):
                        _mm(nc, ps[:], src_kv[ic][:, kc * 128:(kc + 1) * 128],
                            W[f"wv{bi}"][:, ic, :],
                            start=(ic == 0), stop=(ic == DC - 1))
                    if PV_FP8:
                        dst = vtok[kc // 2][:, kc % 2, :, 0:DK]
                    else:
                        dst = vtok[kc][:, :, 0:DK]
                    src = ps[:].rearrange("p (h k) -> p h k", h=H)
                    if kc % 2 == 0:
                        nc.scalar.activation(dst, src,
                                             mybir.ActivationFunctionType.Relu)
                    else:
                        nc.vector.tensor_scalar_max(dst, src, 0.0)

            # ============ attention ===================================
            with ExitStack() as ast:
                pp = ast.enter_context(tc.tile_pool(name=f"pp{bi}", bufs=3))
                psB = ast.enter_context(tc.tile_pool(name=f"psB{bi}", bufs=2, space="PSUM"))
                psPV = ast.enter_context(tc.tile_pool(name=f"psPV{bi}", bufs=1, space="PSUM"))
                for hp in range(4):  # heads (2hp, 2hp+1)
                    g = hp // 2
                    pv = [psPV.tile([VW, QT], F32, tag=f"pv{x}", name=f"pv{x}")
                          for x in range(4)]  # index 2j+qt
                    if PV_FP8:
                        ptiles = {qt: [] for qt in range(NQT)}
                        for kc in range(NKC):
                            # j outer / qt inner: consecutive matmuls share the
                            # stationary operand (LDWEIGHTS is not overlapped
                            # by this compiler, but repeats are skipped)
                            scs = [psB.tile([128, 2, QT], F32, tag="sc", name="sc")
                                   for _ in range(NQT)]
                            for j in range(2):
                                hh = 2 * (hp % 2) + j
                                for qt in range(NQT):
                                    _mm(nc, scs[qt][:, j, :],
                                        kT[g][:, kc * 128:(kc + 1) * 128],
                                        qTz[g][hh][:, qt * QT:(qt + 1) * QT],
                                        start=True, stop=True)
                                    if j == 1:
                                        if kc % 2 == 0:
                                            ptiles[qt].append(pp.tile(
                                                [128, 2, 2, QT], FP8,
                                                tag=f"p4_{qt}", name=f"p4_{qt}"))
                                        nc.scalar.activation(
                                            ptiles[qt][-1][:, :, kc % 2, :],
                                            scs[qt][:],
                                            mybir.ActivationFunctionType.Exp,
                                            scale=SCALE, bias=ebias[:])
                            if kc % 2 == 0 and kc >= 2:
                                pc = kc // 2 - 1
                                for qt in range(NQT):
                                    for j in range(2):
                                        _mm(nc, pv[2 * j + qt][:],
                                            vtok[pc][:, :, 2 * hp + j, 0:VW],
                                            ptiles[qt][pc][:, j, :, :],
                                            start=(pc == 0), stop=False,
                                            perf_mode=mybir.MatmulPerfMode.DoubleRow,
                                            skip_group_check=True)
                        pc = NKC // 2 - 1
                        for qt in range(NQT):
                            for j in range(2):
                                _mm(nc, pv[2 * j + qt][:],
                                    vtok[pc][:, :, 2 * hp + j, 0:VW],
                                    ptiles[qt][pc][:, j, :, :],
                                    start=False, stop=True,
                                    perf_mode=mybir.MatmulPerfMode.DoubleRow,
                                    skip_group_check=True)
                    else:
                        ptiles = {qt: [] for qt in range(NQT)}
                        for kc in range(NKC):
                            for qt in range(NQT):
                                sc = psB.tile([128, 2, QT], F32, tag="sc", name="sc")
                                for j in range(2):
                                    hh = 2 * (hp % 2) + j
                                    _mm(nc, sc[:, j, :],
                                        kT[g][:, kc * 128:(kc + 1) * 128],
                                        qTz[g][hh][:, qt * QT:(qt + 1) * QT],
                                        start=True, stop=True)
                                p2 = pp.tile([128, 2, QT], BF16, tag=f"p2_{qt}", name=f"p2_{qt}")
                                ptiles[qt].append(p2)
                                nc.scalar.activation(
                                    p2[:], sc[:],
                                    mybir.ActivationFunctionType.Exp,
                                    scale=SCALE, bias=ebias[:])
                            if kc >= 1:
                                for qt in range(NQT):
                                    for j in range(2):
                                        _mm(nc, pv[2 * j + qt][:],
                                            vtok[kc - 1][:, 2 * hp + j, :],
                                            ptiles[qt][kc - 1][:, j, :],
                                            start=(kc - 1 == 0), stop=False,
                                            skip_group_check=True)
                        for qt in range(NQT):
                            for j in range(2):
                                _mm(nc, pv[2 * j + qt][:],
                                    vtok[NKC - 1][:, 2 * hp + j, :],
                                    ptiles[qt][NKC - 1][:, j, :],
                                    start=False, stop=True,
                                    skip_group_check=True)
                    for j in range(2):
                        h = 2 * hp + j
                        hh = 2 * (hp % 2) + j
                        for qt in range(NQT):
                            qsl = slice(qt * QT, (qt + 1) * QT)
                            nc.vector.tensor_copy(
                                ot[g][32 * hh:32 * hh + 32, qsl],
                                pv[2 * j + qt][0:DK, :])
                            nc.vector.tensor_copy(
                                den[32 * (h % 4):32 * (h % 4) + 1,
                                    (h // 4) * TOWN + qt * QT:
                                    (h // 4) * TOWN + (qt + 1) * QT],
                                pv[2 * j + qt][DK:DK + 1, :])
                    # normalize group g as soon as its heads are done so the
                    # tail hides under the remaining attention work. g=0's
                    # reciprocal runs on the (idle-during-attention) DVE;
                    # g=1's on the scalar via exp(-ln(den)) right after the
                    # last exp (ln/exp share one ACT table set).
                    if hp == 1 or hp == 3:
                        g = hp // 2
                        dsl = slice(g * TOWN, (g + 1) * TOWN)
                        if hp == 1:
                            nc.vector.reciprocal(den[:, dsl], den[:, dsl])
                        else:
                            nc.scalar.activation(den[:, dsl], den[:, dsl],
                                                 mybir.ActivationFunctionType.Ln)
                            nc.scalar.activation(den[:, dsl], den[:, dsl],
                                                 mybir.ActivationFunctionType.Exp,
                                                 scale=-1.0)
                        for hh in range(4):
                            h = 4 * g + hh
                            _bcast(nc, bc_d[8 * bi + h],
                                   den[32 * (h % 4):32 * (h % 4) + 1, dsl],
                                   rb[g][32 * hh:32 * hh + 32, :])
                        nc.vector.tensor_mul(ot[g][:], ot[g][:], rb[g][:])

            # ============ Wo proj + residual + LN1 ====================
            self_ln(nc, tc, W, f"ga{bi}", f"ba{bi}", ot, src_q, x1,
                    consts, name=f"ln1_{bi}", wt=W[f"wo{bi}"], nch=DC)

            # ============ FFN + residual + LN2 ========================
            with tc.tile_pool(name=f"psD{bi}", bufs=3, space="PSUM") as psD:
                for fc in range(FC):
                    pss = [psD.tile([128, QT], F32, tag="ffn1", name="ffn1")
                           for _ in range(NQT)]
                    for ic in range(DC):
                        for qt in range(NQT):
                            _mm(nc, pss[qt][:], W[f"wf1{bi}"][:, ic, fc * 128:(fc + 1) * 128],
                                x1[ic][:, qt * QT:(qt + 1) * QT],
                                start=(ic == 0), stop=(ic == DC - 1))
                    for qt in range(NQT):
                        dst = hT[:, fc, qt * QT:(qt + 1) * QT]
                        if (fc + qt) % 2 == 0:
                            nc.scalar.activation(
                                dst, pss[qt][:], mybir.ActivationFunctionType.Relu)
                        else:
                            nc.vector.tensor_scalar_max(dst, pss[qt][:], 0.0)
            hT_moving = [hT[:, fc, :] for fc in range(FC)]
            out_tiles = x2own if bi == 0 else xout
            self_ln(nc, tc, W, f"gb{bi}", f"bb{bi}", hT_moving, x1,
                    out_tiles, consts, name=f"ln2_{bi}", wt=W[f"wf2{bi}"], nch=FC)

            # ============ exchange (after block 0 only) ===============
            if bi == 0:
                for qt in range(NQT):
                    for i in range(DC):
                        nc.sync.dma_start(xh_d[qt, i],
                                          x2own[i][:, qt * QT:(qt + 1) * QT])
                    nc.gpsimd.collective_compute(
                        "AllGather", mybir.AluOpType.bypass,
                        replica_groups=GROUPS,
                        ins=[xh_d[qt].flatten()], outs=[xg_d[qt].flatten()])
                for qt in range(NQT):
                    for r in range(2):
                        for i in range(DC):
                            nc.sync.dma_start(
                                xt[i][:, r * TOWN + qt * QT:r * TOWN + (qt + 1) * QT],
                                xg_d[qt, r, i])

        for i in range(DC):  # chunked so each quarter leaves as it is ready
            for qt in range(NQT):
                qsl = slice(qt * QT, (qt + 1) * QT)
                nc.sync.dma_start(out_t[i][:, qsl], xout[i][:, qsl])

    return _legalize_multiwaits(nc)


def self_ln(nc, tc, W, gkey, bkey, moving, resid, out_tiles, consts, name,
            wt, nch):
    """proj the `moving` chunks with `wt`, relu, add `resid`, layer-norm
    with (gamma=W[gkey], beta=W[bkey]) -> out_tiles."""
    from contextlib import ExitStack

    ones32 = consts["ones32"]
    scale_row = consts["scale_row"]
    with ExitStack() as st:
        tmp = st.enter_context(tc.tile_pool(name=f"{name}_tmp", bufs=1))
        psC = st.enter_context(tc.tile_pool(name=f"{name}_ps", bufs=2, space="PSUM"))
        psS = st.enter_context(tc.tile_pool(name=f"{name}_st", bufs=2, space="PSUM"))
        psB = st.enter_context(tc.tile_pool(name=f"{name}_bc", bufs=2, space="PSUM"))

        y = [tmp.tile([128, TOWN], BF16, tag=f"y{i}", name=f"y{i}") for i in range(DC)]
        srows = tmp.tile([1, 2, NQT, QT], BF16, tag="srows", name="srows")
        mu_b, rstd_b = {}, {}
        for oc in range(DC):
            pss = [psC.tile([128, QT], F32, tag="proj", name="proj")
                   for _ in range(NQT)]
            for ic in range(nch):
                for qt in range(NQT):
                    _mm(nc, pss[qt][:], wt[:, ic, oc * 128:(oc + 1) * 128],
                        moving[ic][:, qt * QT:(qt + 1) * QT],
                        start=(ic == 0), stop=(ic == nch - 1))
            for qt in range(NQT):
                qsl = slice(qt * QT, (qt + 1) * QT)
                # y = relu(ps) + resid
                nc.vector.scalar_tensor_tensor(
                    y[oc][:, qsl], pss[qt][:], 0.0, resid[oc][:, qsl],
                    op0=mybir.AluOpType.max, op1=mybir.AluOpType.add)
        for qt in range(NQT):
            qsl = slice(qt * QT, (qt + 1) * QT)
            sum_ps = psS.tile([32, QT], F32, tag="stat", name="stat")
            sq_ps = psS.tile([32, QT], F32, tag="stat", name="stat")
            for oc in range(DC):
                ysq = tmp.tile([128, QT], BF16, tag="ysq", name="ysq", bufs=3)
                if (oc + qt) % 2 == 0:
                    nc.scalar.activation(ysq[:], y[oc][:, qsl],
                                         mybir.ActivationFunctionType.Square)
                else:
                    nc.vector.tensor_mul(ysq[:], y[oc][:, qsl], y[oc][:, qsl])
                _mm(nc, sum_ps[:], ones32, y[oc][:, qsl],
                    start=(oc == 0), stop=(oc == DC - 1), skip_group_check=True)
                _mm(nc, sq_ps[:], ones32, ysq[:],
                    start=(oc == 0), stop=(oc == DC - 1), skip_group_check=True)
            nc.vector.tensor_copy(srows[:, 0, qt, :], sum_ps[0:1, :])
            nc.vector.tensor_copy(srows[:, 1, qt, :], sq_ps[0:1, :])
            # broadcast mean and mean-square across partitions (K=1 matmuls)
            mb = psB.tile([128, QT], F32, tag="bc", name="bc")
            m2 = psB.tile([128, QT], F32, tag="bc", name="bc")
            _mm(nc, mb[:], scale_row, srows[:, 0, qt, :], start=True, stop=True)
            _mm(nc, m2[:], scale_row, srows[:, 1, qt, :], start=True, stop=True)
            # var = m2 - mu^2 ; rstd = 1/sqrt(var + eps)
            msb = tmp.tile([128, QT], F32, tag="msb", name="msb", bufs=2)
            nc.vector.tensor_copy(msb[:], mb[:])
            vb = tmp.tile([128, QT], F32, tag="vb", name="vb", bufs=2)
            nc.vector.tensor_mul(vb[:], msb[:], msb[:])
            nc.vector.tensor_sub(vb[:], m2[:], vb[:])
            # rstd = exp(-0.5*ln(var+eps)); ln/exp share one ACT table set
            sq = tmp.tile([128, QT], F32, tag="sq", name="sq", bufs=2)
            nc.scalar.activation(sq[:], vb[:],
                                 mybir.ActivationFunctionType.Ln,
                                 bias=consts["eps128"])
            rbt = tmp.tile([128, QT], F32, tag="rb2", name="rb2", bufs=2)
            nc.scalar.activation(rbt[:], sq[:],
                                 mybir.ActivationFunctionType.Exp,
                                 scale=-0.5)
            mu_b[qt], rstd_b[qt] = msb, rbt
        for oc in range(DC):
            for qt in range(NQT):
                qsl = slice(qt * QT, (qt + 1) * QT)
                t = tmp.tile([128, QT], F32, tag="t", name="t", bufs=3)
                nc.vector.tensor_sub(t[:], y[oc][:, qsl], mu_b[qt][:])
                nc.vector.scalar_tensor_tensor(
                    t[:], t[:], W[gkey][:, oc, :], rstd_b[qt][:],
                    op0=mybir.AluOpType.mult, op1=mybir.AluOpType.mult)
                nc.vector.tensor_scalar_add(out_tiles[oc][:, qsl], t[:],
                                            W[bkey][:, oc, :])


def _install_profile_hook():
    import sys as _sys
    import types as _types

    if "antenv.axon_hooks" in _sys.modules:
        return
    _sys.path.insert(0, "/root/.axon_site")
    try:
        from trn_agent_boot.trn_boot import _ntff_profile_via_ctypes
        hook = _ntff_profile_via_ctypes("/opt/axon/libaxon_pjrt.so")
    except Exception:
        hook = None
    mod = _types.ModuleType("antenv.axon_hooks")
    mod.get_axon_ntff_profile_hook = lambda: hook
    mod.set_axon_ntff_profile_hook = lambda h: None
    _sys.modules["antenv.axon_hooks"] = mod


# ---------------------------------------------------------------- host side
_NC_CACHE = {}


def _get_nc(debug=False):
    if debug not in _NC_CACHE:
        _NC_CACHE[debug] = build()
    return _NC_CACHE[debug]


def _prep_inputs(x, weights):
    import ml_dtypes
    bf = ml_dtypes.bfloat16
    in_maps = []
    wmats = {}
    for bi, (q, k, v, o, f1, f2) in enumerate(
        (("W11", "W12", "W13", "W14", "Wf11", "Wf21"),
         ("W21", "W22", "W23", "W24", "Wf12", "Wf22"))):
        wmats[f"wq{bi}"] = np.ascontiguousarray(
            weights[q].T.reshape(DC, 128, D)).astype(bf)
        wmats[f"wk{bi}"] = np.ascontiguousarray(
            weights[k].T.reshape(DC, 128, D)).astype(bf)
        wmats[f"wv{bi}"] = np.ascontiguousarray(
            weights[v].T.reshape(DC, 128, D)).astype(bf)
        wmats[f"wo{bi}"] = np.ascontiguousarray(
            weights[o].T.reshape(DC, 128, D)).astype(bf)
        wmats[f"wf1{bi}"] = np.ascontiguousarray(
            weights[f1].T.reshape(DC, 128, DFF)).astype(bf)
        wmats[f"wf2{bi}"] = np.ascontiguousarray(
            weights[f2].T.reshape(FC, 128, D)).astype(bf)
    for bi, (g1, b1, g2, b2) in enumerate(
        (("g1", "b1", "g2", "b2"), ("g3", "b3", "g4", "b4"))):
        wmats[f"ga{bi}"] = np.ascontiguousarray(weights[g1].reshape(DC, 128, 1))
        wmats[f"ba{bi}"] = np.ascontiguousarray(weights[b1].reshape(DC, 128, 1))
        wmats[f"gb{bi}"] = np.ascontiguousarray(weights[g2].reshape(DC, 128, 1))
        wmats[f"bb{bi}"] = np.ascontiguousarray(weights[b2].reshape(DC, 128, 1))
    for c in range(NCORES):
        b, half = c // 2, c % 2
        xb = x[b]
        own = xb[half * TOWN:(half + 1) * TOWN]
        other = xb[(1 - half) * TOWN:(2 - half) * TOWN]
        xcore = np.concatenate([own, other], axis=0)
        xt = np.ascontiguousarray(xcore.T.reshape(DC, 128, S)).astype(bf)
        m = {"xt": xt}
        m.update(wmats)
        in_maps.append(m)
    return in_maps


def kernel(x, W11, W12, W13, W14, W21, W22, W23, W24,
           Wf11, Wf21, Wf12, Wf22,
           g1, b1, g2, b2, g3, b3, g4, b4, _debug=False, _trace=False):
    weights = dict(W11=W11, W12=W12, W13=W13, W14=W14,
                   W21=W21, W22=W22, W23=W23, W24=W24,
                   Wf11=Wf11, Wf21=Wf21, Wf12=Wf12, Wf22=Wf22,
                   g1=g1, b1=b1, g2=g2, b2=b2, g3=g3, b3=b3, g4=g4, b4=b4)
    weights = {k: np.asarray(v, dtype=np.float32) for k, v in weights.items()}
    x = np.asarray(x, dtype=np.float32)
    if _trace:
        _install_profile_hook()
    nc = _get_nc(_debug)
    in_maps = _prep_inputs(x, weights)
    res = run_bass_kernel_spmd(nc, in_maps, core_ids=list(range(NCORES)),
                               trace=_trace)
    out = np.empty((B, S, D), dtype=np.float32)
    for c in range(NCORES):
        b, half = c // 2, c % 2
        ot = res.results[c]["out_t"].astype(np.float32).reshape(D, TOWN)
        out[b, half * TOWN:(half + 1) * TOWN] = ot.T
    if _debug or _trace:
        kernel.last_result = res
    return out


# revision 17
# speedup vs baseline: 1.2537x; 1.0044x over previous
"""Two-block transformer encoder (B=4, S=2048, D=256, H=8, DFF=1024) on 8
Trainium2 NeuronCores.

Sharding: core c -> batch b = c//2, sequence half = c%2 (1024 tokens owned).
Weights replicated. Block 1 computes K/V over the full sequence and
Q/FFN/LN over the owned half. Between blocks the halves are exchanged
with chunked AllGathers (bf16) that overlap block-2's Q projection.

Changes vs the original baseline (631us -> ~475us):
- everything bf16 (activations, weights, exchange); PSUM stays fp32
- scores as plain full-contraction matmuls: packed K stationary (shared
  by all four (head, qt) matmuls of a key chunk -- LDWEIGHTS is not
  overlapped by this compiler but repeated stationaries are skipped)
  against zero-padded per-head Q moving tiles. tile_position matmuls
  measured ~2.6x slowdown on adjacent full-array matmuls -- avoided.
- PV in fp8e4 DoubleRow (P written by exp directly as fp8 into kc-pair
  interleaved slots; V relu'd into fp8 pair tiles): 256-key contraction
  halves the PV matmul count. PV emission skewed one kc pair behind
  scores so the strict-FIFO PE never stalls waiting on exp.
- softmax denominators: 1/den via DVE reciprocal for head group 0
  (hides under the second half of attention) and scalar exp(-ln(den))
  for group 1; LN rstd = exp(-0.5*ln(var+eps)) -- ln/exp share one ACT
  table set, and the DVE's iterative reciprocal is ~10 cycles/element
- chunked bf16 AllGather overlapped with block-2 Q projection
- chunked input/output DMAs; relu/square balanced between the scalar
  and vector engines per phase
"""

import numpy as np

import concourse.bass as bass
import concourse.mybir as mybir
import concourse.tile as tile
from concourse.bass_utils import run_bass_kernel_spmd

# ---------------------------------------------------------------- constants
B, S, D, H, DK, DFF = 4, 2048, 256, 8, 32, 1024
NCORES = 8
TOWN = S // 2
QT = 512
NQT = TOWN // QT  # 2
NKC = S // 128  # 16
DC = D // 128  # 2
FC = DFF // 128  # 8
EPS = 1e-5
SCALE = float(1.0 / np.sqrt(np.float32(DK)))
F32 = mybir.dt.float32
BF16 = mybir.dt.bfloat16
FP8 = mybir.dt.float8e4
GROUPS = [[0, 1], [2, 3], [4, 5], [6, 7]]
VW = DK + 1  # V columns per head incl. ones column
PVW = 36  # padded V row pitch (fp8 DoubleRow needs 16B-aligned strides)

PV_FP8 = True  # fp8e4 DoubleRow PV path
EXP_BIAS = -1.0 if PV_FP8 else 0.0  # headroom below fp8e4 max; cancels in softmax


def _legalize_multiwaits(nc):
    """Split multi-wait instructions into prefix EventSemaphore waits."""
    import json

    orig = nc.to_json_bytes

    def patched():
        j = json.loads(orig())
        n = 0
        for fn in j.get("functions", []):
            for bb in fn.get("blocks", []):
                out = []
                for ins in bb.get("instructions", []):
                    si = ins.get("sync_info") or {}
                    waits = si.get("on_wait") or []
                    if len(waits) > 1:
                        for w in waits[:-1]:
                            n += 1
                            out.append({
                                "engine": ins["engine"],
                                "ins": [],
                                "name": f"I-mwsplit-{n}",
                                "opcode": "EventSemaphore",
                                "outs": [],
                                "sync_info": {"on_update": [], "on_wait": [w]},
                            })
                        si["on_wait"] = [waits[-1]]
                    out.append(ins)
                bb["instructions"] = out
        return json.dumps(j).encode()

    nc.to_json_bytes = patched
    return nc


def _mm(nc, out, lhsT, rhs, **kw):
    nc.tensor.matmul(out, lhsT, rhs, **kw)


def _bcast(nc, bc_row, src_row, out_ap):
    """Broadcast a [1, N] SBUF row across partitions via a DRAM bounce."""
    n = src_row.shape[-1]
    row = bc_row[0:n]
    nc.gpsimd.dma_start(row, src_row)
    bcast = bass.AP(tensor=row.tensor, offset=row.offset,
                    ap=[[0, out_ap.shape[0]], *[list(d) for d in row.ap]])
    nc.gpsimd.dma_start(out_ap, bcast)


def build():
    from contextlib import ExitStack

    nc = bass.Bass(num_devices=NCORES)

    xt_in = nc.dram_tensor("xt", [DC, 128, S], BF16, kind="ExternalInput")
    wd = {}
    for bi in range(2):
        for nm in ("wq", "wk", "wv", "wo"):
            wd[f"{nm}{bi}"] = nc.dram_tensor(f"{nm}{bi}", [DC, 128, D], BF16, kind="ExternalInput")
        wd[f"wf1{bi}"] = nc.dram_tensor(f"wf1{bi}", [DC, 128, DFF], BF16, kind="ExternalInput")
        wd[f"wf2{bi}"] = nc.dram_tensor(f"wf2{bi}", [FC, 128, D], BF16, kind="ExternalInput")
        for nm in ("ga", "ba", "gb", "bb"):
            wd[f"{nm}{bi}"] = nc.dram_tensor(f"{nm}{bi}", [DC, 128, 1], F32, kind="ExternalInput")
    out_t = nc.dram_tensor("out_t", [DC, 128, TOWN], F32, kind="ExternalOutput")
    xh_d = nc.dram_tensor("xh_d", [NQT, DC, 128, QT], BF16)
    xg_d = nc.dram_tensor("xg_d", [NQT, 2, DC, 128, QT], BF16)
    bc_d = nc.dram_tensor("bc_d", [16, TOWN], F32)

    with tile.TileContext(nc) as tc, ExitStack() as top:
        top.enter_context(nc.allow_low_precision(
            reason="bf16/fp8 activations; matmul accumulation stays fp32 in PSUM"))
        persist = top.enter_context(tc.tile_pool(name="persist", bufs=1))

        ones32 = persist.tile([128, 32], BF16, tag="ones32", name="ones32")
        nc.vector.memset(ones32, 1.0)
        scale_row = persist.tile([1, 128], BF16, tag="scale_row", name="scale_row")
        nc.vector.memset(scale_row, 1.0 / D)
        eps128 = persist.tile([128, 1], F32, tag="eps128", name="eps128")
        nc.vector.memset(eps128, EPS)
        ebias = persist.tile([128, 1], F32, tag="ebias", name="ebias")
        nc.vector.memset(ebias, EXP_BIAS)
        consts = {"ones32": ones32, "scale_row": scale_row, "eps128": eps128}

        # ---- block input first (unblocks QKV quickly), then weights
        xt = [persist.tile([128, S], BF16, tag=f"xt{i}", name=f"xt{i}") for i in range(DC)]
        for i in range(DC):  # own half first: unblocks the q projection early
            nc.sync.dma_start(xt[i][:, 0:TOWN], xt_in[i][:, 0:TOWN])
        for i in range(DC):
            nc.sync.dma_start(xt[i][:, TOWN:S], xt_in[i][:, TOWN:S])
        W = {}
        for bi in range(2):
            for nm, chunks, width in (
                ("wq", DC, D), ("wk", DC, D), ("wv", DC, D), ("wo", DC, D),
                ("wf1", DC, DFF), ("wf2", FC, D),
            ):
                t = persist.tile([128, chunks, width], BF16, tag=f"{nm}{bi}", name=f"{nm}{bi}")
                for c in range(chunks):
                    nc.sync.dma_start(t[:, c, :], wd[f"{nm}{bi}"][c])
                W[f"{nm}{bi}"] = t
            for nm in ("ga", "ba", "gb", "bb"):
                t = persist.tile([128, DC, 1], F32, tag=f"{nm}{bi}", name=f"{nm}{bi}")
                for c in range(DC):
                    nc.sync.dma_start(t[:, c, :], wd[f"{nm}{bi}"][c])
                W[f"{nm}{bi}"] = t

        # persistent activation tiles (reused across both blocks)
        # zero-padded per-head Q (moving side): only band hh nonzero, so
        # scores are plain full-contraction matmuls (tile_position matmuls
        # slow down adjacent full-array matmuls ~2.6x; measured) and all four
        # (j, qt) score matmuls of a kc share one packed-K stationary
        # (LDWEIGHTS is not overlapped by this compiler; repeats are skipped).
        qTz = [[persist.tile([128, TOWN], BF16, tag=f"qTz{g}_{hh}", name=f"qTz{g}_{hh}")
                for hh in range(4)] for g in range(DC)]
        for g in range(DC):
            for hh in range(4):
                nc.vector.memset(qTz[g][hh], 0.0)
        kT = [persist.tile([128, S], BF16, tag=f"kT{g}", name=f"kT{g}") for g in range(DC)]
        if PV_FP8:
            vtok = [persist.tile([128, 2, H, PVW], FP8, tag=f"vt{p}", name=f"vt{p}")
                    for p in range(NKC // 2)]
            for p in range(NKC // 2):
                nc.vector.memset(vtok[p][:, :, :, DK:DK + 1], 1.0)
        else:
            vtok = [persist.tile([128, H, VW], BF16, tag=f"vt{k}", name=f"vt{k}")
                    for k in range(NKC)]
            for k in range(NKC):
                nc.vector.memset(vtok[k][:, :, DK:VW], 1.0)
        ot = [persist.tile([128, TOWN], BF16, tag=f"ot{g}", name=f"ot{g}") for g in range(DC)]
        x1 = [persist.tile([128, TOWN], BF16, tag=f"x1_{i}", name=f"x1_{i}") for i in range(DC)]
        hT = persist.tile([128, FC, TOWN], BF16, tag="hT", name="hT")
        x2own = [persist.tile([128, TOWN], BF16, tag=f"x2own{i}", name=f"x2own{i}")
                 for i in range(DC)]
        xout = [persist.tile([128, TOWN], F32, tag=f"xout{i}", name=f"xout{i}")
                for i in range(DC)]
        den = persist.tile([128, 2 * TOWN], F32, tag="den", name="den")
        rb = [persist.tile([128, TOWN], F32, tag=f"rb{g}", name=f"rb{g}") for g in range(DC)]

        for bi in range(2):
            src_q = [xt[i][:, 0:TOWN] for i in range(DC)] if bi == 0 else \
                    [x2own[i][:] for i in range(DC)]
            src_kv = xt
            nc.gpsimd.memset(den[:], 1.0)

            # ============ QKV projections =============================
            with tc.tile_pool(name=f"psA{bi}", bufs=4, space="PSUM") as psA:
                # q^T, own tokens only
                for oc in range(DC):
                    pss = [psA.tile([128, QT], F32, tag="qkv", name="qkv")
                           for _ in range(NQT)]
                    for ic in range(DC):
                        for st_i in range(NQT):
                            _mm(nc, pss[st_i][:], W[f"wq{bi}"][:, ic, oc * 128:(oc + 1) * 128],
                                src_q[ic][:, st_i * QT:(st_i + 1) * QT],
                                start=(ic == 0), stop=(ic == DC - 1))
                    for st_i in range(NQT):
                        for hh in range(4):
                            dst = qTz[oc][hh][32 * hh:32 * hh + 32,
                                              st_i * QT:(st_i + 1) * QT]
                            src = pss[st_i][32 * hh:32 * hh + 32, :]
                            if (st_i + hh) % 2 == 0:
                                nc.scalar.activation(dst, src,
                                                     mybir.ActivationFunctionType.Relu)
                            else:
                                nc.vector.tensor_scalar_max(dst, src, 0.0)
                # k^T over full sequence (packed; heads on 32-row bands)
                for oc in range(DC):
                    pss = [psA.tile([128, QT], F32, tag="qkv", name="qkv")
                           for _ in range(S // QT)]
                    for ic in range(DC):
                        for st_i in range(S // QT):
                            _mm(nc, pss[st_i][:], W[f"wk{bi}"][:, ic, oc * 128:(oc + 1) * 128],
                                src_kv[ic][:, st_i * QT:(st_i + 1) * QT],
                                start=(ic == 0), stop=(ic == DC - 1))
                    for st_i in range(S // QT):
                        dst = kT[oc][:, st_i * QT:(st_i + 1) * QT]
                        if st_i % 2 == 0:
                            nc.scalar.activation(dst, pss[st_i][:],
                                                 mybir.ActivationFunctionType.Relu)
                        else:
                            nc.vector.tensor_scalar_max(dst, pss[st_i][:], 0.0)
                # token-major V with ones column, full sequence
                for kc in range(NKC):
                    ps = psA.tile([128, D], F32, tag="vtok", name="vtok")
                    for ic in range(DC):
                        _mm(nc, ps[:], src_kv[ic][:, kc * 128:(kc + 1) * 128],
                            W[f"wv{bi}"][:, ic, :],
                            start=(ic == 0), stop=(ic == DC - 1))
                    if PV_FP8:
                        dst = vtok[kc // 2][:, kc % 2, :, 0:DK]
                    else:
                        dst = vtok[kc][:, :, 0:DK]
                    src = ps[:].rearrange("p (h k) -> p h k", h=H)
                    if kc % 2 == 0:
                        nc.scalar.activation(dst, src,
                                             mybir.ActivationFunctionType.Relu)
                    else:
                        nc.vector.tensor_scalar_max(dst, src, 0.0)

            # ============ attention ===================================
            with ExitStack() as ast:
                pp = ast.enter_context(tc.tile_pool(name=f"pp{bi}", bufs=3))
                psB = ast.enter_context(tc.tile_pool(name=f"psB{bi}", bufs=2, space="PSUM"))
                psPV = ast.enter_context(tc.tile_pool(name=f"psPV{bi}", bufs=1, space="PSUM"))
                for hp in range(4):  # heads (2hp, 2hp+1)
                    g = hp // 2
                    pv = [psPV.tile([VW, QT], F32, tag=f"pv{x}", name=f"pv{x}")
                          for x in range(4)]  # index 2j+qt
                    if PV_FP8:
                        ptiles = {qt: [] for qt in range(NQT)}
                        for kc in range(NKC):
                            # j outer / qt inner: consecutive matmuls share the
                            # stationary operand (LDWEIGHTS is not overlapped
                            # by this compiler, but repeats are skipped)
                            scs = [psB.tile([128, 2, QT], F32, tag="sc", name="sc")
                                   for _ in range(NQT)]
                            for j in range(2):
                                hh = 2 * (hp % 2) + j
                                for qt in range(NQT):
                                    _mm(nc, scs[qt][:, j, :],
                                        kT[g][:, kc * 128:(kc + 1) * 128],
                                        qTz[g][hh][:, qt * QT:(qt + 1) * QT],
                                        start=True, stop=True)
                                    if j == 1:
                                        if kc % 2 == 0:
                                            ptiles[qt].append(pp.tile(
                                                [128, 2, 2, QT], FP8,
                                                tag=f"p4_{qt}", name=f"p4_{qt}"))
                                        nc.scalar.activation(
                                            ptiles[qt][-1][:, :, kc % 2, :],
                                            scs[qt][:],
                                            mybir.ActivationFunctionType.Exp,
                                            scale=SCALE, bias=ebias[:])
                            if kc % 2 == 0 and kc >= 2:
                                pc = kc // 2 - 1
                                for qt in range(NQT):
                                    for j in range(2):
                                        _mm(nc, pv[2 * j + qt][:],
                                            vtok[pc][:, :, 2 * hp + j, 0:VW],
                                            ptiles[qt][pc][:, j, :, :],
                                            start=(pc == 0), stop=False,
                                            perf_mode=mybir.MatmulPerfMode.DoubleRow,
                                            skip_group_check=True)
                        pc = NKC // 2 - 1
                        for qt in range(NQT):
                            for j in range(2):
                                _mm(nc, pv[2 * j + qt][:],
                                    vtok[pc][:, :, 2 * hp + j, 0:VW],
                                    ptiles[qt][pc][:, j, :, :],
                                    start=False, stop=True,
                                    perf_mode=mybir.MatmulPerfMode.DoubleRow,
                                    skip_group_check=True)
                    else:
                        ptiles = {qt: [] for qt in range(NQT)}
                        for kc in range(NKC):
                            for qt in range(NQT):
                                sc = psB.tile([128, 2, QT], F32, tag="sc", name="sc")
                                for j in range(2):
                                    hh = 2 * (hp % 2) + j
                                    _mm(nc, sc[:, j, :],
                                        kT[g][:, kc * 128:(kc + 1) * 128],
                                        qTz[g][hh][:, qt * QT:(qt + 1) * QT],
                                        start=True, stop=True)
                                p2 = pp.tile([128, 2, QT], BF16, tag=f"p2_{qt}", name=f"p2_{qt}")
                                ptiles[qt].append(p2)
                                nc.scalar.activation(
                                    p2[:], sc[:],
                                    mybir.ActivationFunctionType.Exp,
                                    scale=SCALE, bias=ebias[:])
                            if kc >= 1:
                                for qt in range(NQT):
                                    for j in range(2):
                                        _mm(nc, pv[2 * j + qt][:],
                                            vtok[kc - 1][:, 2 * hp + j, :],
                                            ptiles[qt][kc - 1][:, j, :],
                                            start=(kc - 1 == 0), stop=False,
                                            skip_group_check=True)
                        for qt in range(NQT):
                            for j in range(2):
                                _mm(nc, pv[2 * j + qt][:],
                                    vtok[NKC - 1][:, 2 * hp + j, :],
                                    ptiles[qt][NKC - 1][:, j, :],
                                    start=False, stop=True,
                                    skip_group_check=True)
                    for j in range(2):
                        h = 2 * hp + j
                        hh = 2 * (hp % 2) + j
                        for qt in range(NQT):
                            qsl = slice(qt * QT, (qt + 1) * QT)
                            nc.vector.tensor_copy(
                                ot[g][32 * hh:32 * hh + 32, qsl],
                                pv[2 * j + qt][0:DK, :])
                            nc.vector.tensor_copy(
                                den[32 * (h % 4):32 * (h % 4) + 1,
                                    (h // 4) * TOWN + qt * QT:
                                    (h // 4) * TOWN + (qt + 1) * QT],
                                pv[2 * j + qt][DK:DK + 1, :])
                    # normalize group g as soon as its heads are done so the
                    # tail hides under the remaining attention work. g=0's
                    # reciprocal runs on the (idle-during-attention) DVE;
                    # g=1's on the scalar via exp(-ln(den)) right after the
                    # last exp (ln/exp share one ACT table set).
                    if hp == 1 or hp == 3:
                        g = hp // 2
                        dsl = slice(g * TOWN, (g + 1) * TOWN)
                        if hp == 1:
                            nc.vector.reciprocal(den[:, dsl], den[:, dsl])
                        else:
                            nc.scalar.activation(den[:, dsl], den[:, dsl],
                                                 mybir.ActivationFunctionType.Ln)
                            nc.scalar.activation(den[:, dsl], den[:, dsl],
                                                 mybir.ActivationFunctionType.Exp,
                                                 scale=-1.0)
                        for hh in range(4):
                            h = 4 * g + hh
                            _bcast(nc, bc_d[8 * bi + h],
                                   den[32 * (h % 4):32 * (h % 4) + 1, dsl],
                                   rb[g][32 * hh:32 * hh + 32, :])
                        nc.vector.tensor_mul(ot[g][:], ot[g][:], rb[g][:])

            # ============ Wo proj + residual + LN1 ====================
            self_ln(nc, tc, W, f"ga{bi}", f"ba{bi}", ot, src_q, x1,
                    consts, name=f"ln1_{bi}", wt=W[f"wo{bi}"], nch=DC)

            # ============ FFN + residual + LN2 ========================
            with tc.tile_pool(name=f"psD{bi}", bufs=3, space="PSUM") as psD:
                for fc in range(FC):
                    pss = [psD.tile([128, QT], F32, tag="ffn1", name="ffn1")
                           for _ in range(NQT)]
                    for ic in range(DC):
                        for qt in range(NQT):
                            _mm(nc, pss[qt][:], W[f"wf1{bi}"][:, ic, fc * 128:(fc + 1) * 128],
                                x1[ic][:, qt * QT:(qt + 1) * QT],
                                start=(ic == 0), stop=(ic == DC - 1))
                    for qt in range(NQT):
                        dst = hT[:, fc, qt * QT:(qt + 1) * QT]
                        if (fc + qt) % 2 == 0:
                            nc.scalar.activation(
                                dst, pss[qt][:], mybir.ActivationFunctionType.Relu)
                        else:
                            nc.vector.tensor_scalar_max(dst, pss[qt][:], 0.0)
            hT_moving = [hT[:, fc, :] for fc in range(FC)]
            out_tiles = x2own if bi == 0 else xout
            self_ln(nc, tc, W, f"gb{bi}", f"bb{bi}", hT_moving, x1,
                    out_tiles, consts, name=f"ln2_{bi}", wt=W[f"wf2{bi}"], nch=FC)

            # ============ exchange (after block 0 only) ===============
            if bi == 0:
                for qt in range(NQT):
                    for i in range(DC):
                        nc.sync.dma_start(xh_d[qt, i],
                                          x2own[i][:, qt * QT:(qt + 1) * QT])
                    nc.gpsimd.collective_compute(
                        "AllGather", mybir.AluOpType.bypass,
                        replica_groups=GROUPS,
                        ins=[xh_d[qt].flatten()], outs=[xg_d[qt].flatten()])
                for qt in range(NQT):
                    for r in range(2):
                        for i in range(DC):
                            nc.sync.dma_start(
                                xt[i][:, r * TOWN + qt * QT:r * TOWN + (qt + 1) * QT],
                                xg_d[qt, r, i])

        for i in range(DC):  # chunked so each quarter leaves as it is ready
            for qt in range(NQT):
                qsl = slice(qt * QT, (qt + 1) * QT)
                nc.sync.dma_start(out_t[i][:, qsl], xout[i][:, qsl])

    return _legalize_multiwaits(nc)


def self_ln(nc, tc, W, gkey, bkey, moving, resid, out_tiles, consts, name,
            wt, nch):
    """proj the `moving` chunks with `wt`, relu, add `resid`, layer-norm
    with (gamma=W[gkey], beta=W[bkey]) -> out_tiles."""
    from contextlib import ExitStack

    ones32 = consts["ones32"]
    scale_row = consts["scale_row"]
    with ExitStack() as st:
        tmp = st.enter_context(tc.tile_pool(name=f"{name}_tmp", bufs=1))
        psC = st.enter_context(tc.tile_pool(name=f"{name}_ps", bufs=2, space="PSUM"))
        psS = st.enter_context(tc.tile_pool(name=f"{name}_st", bufs=2, space="PSUM"))
        psB = st.enter_context(tc.tile_pool(name=f"{name}_bc", bufs=2, space="PSUM"))

        y = [tmp.tile([128, TOWN], BF16, tag=f"y{i}", name=f"y{i}") for i in range(DC)]
        srows = tmp.tile([1, 2, NQT, QT], BF16, tag="srows", name="srows")
        mu_b, rstd_b = {}, {}
        for oc in range(DC):
            pss = [psC.tile([128, QT], F32, tag="proj", name="proj")
                   for _ in range(NQT)]
            for ic in range(nch):
                for qt in range(NQT):
                    _mm(nc, pss[qt][:], wt[:, ic, oc * 128:(oc + 1) * 128],
                        moving[ic][:, qt * QT:(qt + 1) * QT],
                        start=(ic == 0), stop=(ic == nch - 1))
            for qt in range(NQT):
                qsl = slice(qt * QT, (qt + 1) * QT)
                # y = relu(ps) + resid
                nc.vector.scalar_tensor_tensor(
                    y[oc][:, qsl], pss[qt][:], 0.0, resid[oc][:, qsl],
                    op0=mybir.AluOpType.max, op1=mybir.AluOpType.add)
        for qt in range(NQT):
            qsl = slice(qt * QT, (qt + 1) * QT)
            sum_ps = psS.tile([32, QT], F32, tag="stat", name="stat")
            sq_ps = psS.tile([32, QT], F32, tag="stat", name="stat")
            for oc in range(DC):
                ysq = tmp.tile([128, QT], BF16, tag="ysq", name="ysq", bufs=3)
                if (oc + qt) % 2 == 0:
                    nc.scalar.activation(ysq[:], y[oc][:, qsl],
                                         mybir.ActivationFunctionType.Square)
                else:
                    nc.vector.tensor_mul(ysq[:], y[oc][:, qsl], y[oc][:, qsl])
                _mm(nc, sum_ps[:], ones32, y[oc][:, qsl],
                    start=(oc == 0), stop=(oc == DC - 1), skip_group_check=True)
                _mm(nc, sq_ps[:], ones32, ysq[:],
                    start=(oc == 0), stop=(oc == DC - 1), skip_group_check=True)
            nc.vector.tensor_copy(srows[:, 0, qt, :], sum_ps[0:1, :])
            nc.vector.tensor_copy(srows[:, 1, qt, :], sq_ps[0:1, :])
            # broadcast mean and mean-square across partitions (K=1 matmuls)
            mb = psB.tile([128, QT], F32, tag="bc", name="bc")
            m2 = psB.tile([128, QT], F32, tag="bc", name="bc")
            _mm(nc, mb[:], scale_row, srows[:, 0, qt, :], start=True, stop=True)
            _mm(nc, m2[:], scale_row, srows[:, 1, qt, :], start=True, stop=True)
            # var = m2 - mu^2 ; rstd = 1/sqrt(var + eps)
            msb = tmp.tile([128, QT], F32, tag="msb", name="msb", bufs=2)
            nc.vector.tensor_copy(msb[:], mb[:])
            vb = tmp.tile([128, QT], F32, tag="vb", name="vb", bufs=2)
            nc.vector.tensor_mul(vb[:], msb[:], msb[:])
            nc.vector.tensor_sub(vb[:], m2[:], vb[:])
            # rstd = exp(-0.5*ln(var+eps)); ln/exp share one ACT table set
            sq = tmp.tile([128, QT], F32, tag="sq", name="sq", bufs=2)
            nc.scalar.activation(sq[:], vb[:],
                                 mybir.ActivationFunctionType.Ln,
                                 bias=consts["eps128"])
            rbt = tmp.tile([128, QT], F32, tag="rb2", name="rb2", bufs=2)
            nc.scalar.activation(rbt[:], sq[:],
                                 mybir.ActivationFunctionType.Exp,
                                 scale=-0.5)
            mu_b[qt], rstd_b[qt] = msb, rbt
        for oc in range(DC):
            for qt in range(NQT):
                qsl = slice(qt * QT, (qt + 1) * QT)
                t = tmp.tile([128, QT], F32, tag="t", name="t", bufs=3)
                nc.vector.tensor_sub(t[:], y[oc][:, qsl], mu_b[qt][:])
                nc.vector.scalar_tensor_tensor(
                    t[:], t[:], W[gkey][:, oc, :], rstd_b[qt][:],
                    op0=mybir.AluOpType.mult, op1=mybir.AluOpType.mult)
                nc.vector.tensor_scalar_add(out_tiles[oc][:, qsl], t[:],
                                            W[bkey][:, oc, :])


def _install_profile_hook():
    import sys as _sys
    import types as _types

    if "antenv.axon_hooks" in _sys.modules:
        return
    _sys.path.insert(0, "/root/.axon_site")
    try:
        from trn_agent_boot.trn_boot import _ntff_profile_via_ctypes
        hook = _ntff_profile_via_ctypes("/opt/axon/libaxon_pjrt.so")
    except Exception:
        hook = None
    mod = _types.ModuleType("antenv.axon_hooks")
    mod.get_axon_ntff_profile_hook = lambda: hook
    mod.set_axon_ntff_profile_hook = lambda h: None
    _sys.modules["antenv.axon_hooks"] = mod


# ---------------------------------------------------------------- host side
_NC_CACHE = {}


def _get_nc(debug=False):
    if debug not in _NC_CACHE:
        _NC_CACHE[debug] = build()
    return _NC_CACHE[debug]


def _prep_inputs(x, weights):
    import ml_dtypes
    bf = ml_dtypes.bfloat16
    in_maps = []
    wmats = {}
    for bi, (q, k, v, o, f1, f2) in enumerate(
        (("W11", "W12", "W13", "W14", "Wf11", "Wf21"),
         ("W21", "W22", "W23", "W24", "Wf12", "Wf22"))):
        wmats[f"wq{bi}"] = np.ascontiguousarray(
            weights[q].T.reshape(DC, 128, D)).astype(bf)
        wmats[f"wk{bi}"] = np.ascontiguousarray(
            weights[k].T.reshape(DC, 128, D)).astype(bf)
        wmats[f"wv{bi}"] = np.ascontiguousarray(
            weights[v].T.reshape(DC, 128, D)).astype(bf)
        wmats[f"wo{bi}"] = np.ascontiguousarray(
            weights[o].T.reshape(DC, 128, D)).astype(bf)
        wmats[f"wf1{bi}"] = np.ascontiguousarray(
            weights[f1].T.reshape(DC, 128, DFF)).astype(bf)
        wmats[f"wf2{bi}"] = np.ascontiguousarray(
            weights[f2].T.reshape(FC, 128, D)).astype(bf)
    for bi, (g1, b1, g2, b2) in enumerate(
        (("g1", "b1", "g2", "b2"), ("g3", "b3", "g4", "b4"))):
        wmats[f"ga{bi}"] = np.ascontiguousarray(weights[g1].reshape(DC, 128, 1))
        wmats[f"ba{bi}"] = np.ascontiguousarray(weights[b1].reshape(DC, 128, 1))
        wmats[f"gb{bi}"] = np.ascontiguousarray(weights[g2].reshape(DC, 128, 1))
        wmats[f"bb{bi}"] = np.ascontiguousarray(weights[b2].reshape(DC, 128, 1))
    for c in range(NCORES):
        b, half = c // 2, c % 2
        xb = x[b]
        own = xb[half * TOWN:(half + 1) * TOWN]
        other = xb[(1 - half) * TOWN:(2 - half) * TOWN]
        xcore = np.concatenate([own, other], axis=0)
        xt = np.ascontiguousarray(xcore.T.reshape(DC, 128, S)).astype(bf)
        m = {"xt": xt}
        m.update(wmats)
        in_maps.append(m)
    return in_maps


def kernel(x, W11, W12, W13, W14, W21, W22, W23, W24,
           Wf11, Wf21, Wf12, Wf22,
           g1, b1, g2, b2, g3, b3, g4, b4, _debug=False, _trace=False):
    weights = dict(W11=W11, W12=W12, W13=W13, W14=W14,
                   W21=W21, W22=W22, W23=W23, W24=W24,
                   Wf11=Wf11, Wf21=Wf21, Wf12=Wf12, Wf22=Wf22,
                   g1=g1, b1=b1, g2=g2, b2=b2, g3=g3, b3=b3, g4=g4, b4=b4)
    weights = {k: np.asarray(v, dtype=np.float32) for k, v in weights.items()}
    x = np.asarray(x, dtype=np.float32)
    if _trace:
        _install_profile_hook()
    nc = _get_nc(_debug)
    in_maps = _prep_inputs(x, weights)
    res = run_bass_kernel_spmd(nc, in_maps, core_ids=list(range(NCORES)),
                               trace=_trace)
    out = np.empty((B, S, D), dtype=np.float32)
    for c in range(NCORES):
        b, half = c // 2, c % 2
        ot = res.results[c]["out_t"].astype(np.float32).reshape(D, TOWN)
        out[b, half * TOWN:(half + 1) * TOWN] = ot.T
    if _debug or _trace:
        kernel.last_result = res
    return out
